# revision 1
# baseline (speedup 1.0000x reference)
"""Trainium2 Bass kernel for nn_Coefficients: assemble the MNA coefficient
block matrix  [[M, 0, 0], [0, I, -M^T], [diag(z), diag(y), 0]]  of shape
[N+2E, 2E+N] from M [N,E], params/kinds/sw_params.

Sharding (8 cores, SPMD — one program, per-core data):
  core c owns kcl rows [128c,128c+128), kvl rows e in [256c,256c+256) and
  elem rows e in the same range.  Each core writes a local out_main
  [640, 5120] (its kcl/kvl/elem row chunks, with zeros where the
  core-dependent diagonal bands go) plus out_bands [768, 256]: the three
  256x256 diagonal blocks (identity, diag(z), diag(y)) whose global column
  position depends on the core; the host unshard step places rows and
  overlays bands into the full [5120, 5120] output.

The toolchain allows only one sync-wait per instruction, so the kernel is
structured as <=8 HWDGE DMAs (no DMA sem-lane reuse) each with at most one
producer dependency.
"""

import numpy as np

N, E, SIG = 1024, 2048, 64
C = 8            # cores
RK = N // C      # 128 kcl rows per core
RE = E // C      # 256 kvl/elem rows per core
W = 2 * E + N    # 5120 output width
DT = 1e-6

_cache = {}


def _build_nc():
    import concourse.bass as bass
    import concourse.mybir as mybir
    from concourse.tile import TileContext, add_dep_helper

    f32 = mybir.dt.float32
    nc = bass.Bass(name="coeffs_scatter", enable_partition_id=False)

    mrow = nc.dram_tensor("mrow", [RK, E], f32, kind="ExternalInput")
    negmt = nc.dram_tensor("negmt", [RE, N], f32, kind="ExternalInput")
    # Diagonal values [128, 4]: cols (z0, z1, y0, y1); col k holds
    # vals[128*(k%2) + p] at row p.  Broadcast on-chip via step-0 APs.
    vb = nc.dram_tensor("vb", [128, 4], f32, kind="ExternalInput")

    out_main = nc.dram_tensor("out_main", [RK + 2 * RE, W], f32, kind="ExternalOutput")
    # Six [128, 256] half-bands (i0 i1 z0 z1 y0 y1) packed along the free
    # dim — SBUF layout dumped verbatim so the DMA gets 6 KB descriptors;
    # the host unpacks.
    out_bands = nc.dram_tensor("out_bands", [128, 6 * RE], f32, kind="ExternalOutput")

    with TileContext(nc) as tc:
        with tc.tile_pool(name="pool", bufs=1) as pool:
            # Band value load first on the SP ring (small; its consumers are
            # the affine_selects feeding the band DMA).  The order-only dep
            # keeps the scheduler from putting mrow ahead of it in the FIFO,
            # which would delay vbt's completion (and the selects) by ~6 us.
            vbt = pool.tile([128, 4], f32, tag="vbt")
            vbt_dma = nc.sync.dma_start(out=vbt[:], in_=vb[:, :])

            # Big DRAM->DRAM copies: M rows into the kcl block, -M^T rows
            # into the kvl right block.  No deps, start immediately.
            mrow_dma = nc.sync.dma_start(out=out_main[0:RK, 0:E], in_=mrow[:, :])
            nc.scalar.dma_start(out=out_main[RK:RK + RE, 2 * E:W], in_=negmt[:, :])
            add_dep_helper(mrow_dma.ins, vbt_dma.ins, sync=False,
                           reason="keep vbt first in the SP FIFO")

            # Zero source tile, read repeatedly (broadcast AP) by the
            # zero-fill DMAs.  Full output width so zero-fill descriptors
            # stay large (20 KB).  The memset gates the zero fills, so it is
            # split across DVE and GpSimd to halve the gate.
            zt = pool.tile([128, W], f32, tag="zt")
            nc.vector.memset(zt[:, 0:W // 2], 0.0)
            nc.gpsimd.memset(zt[:, W // 2:W], 0.0)

            ones = pool.tile([128, 1], f32, tag="ones")
            nc.vector.memset(ones[:], 1.0)

            # Zero fills: one DMA per block region; 256-row regions use a
            # 3D AP with the 128-row chunk index broadcast on the zt side.
            def zfill(engine, row0, nrows, col0, width):
                k = nrows // 128
                dst = out_main[row0:row0 + nrows, col0:col0 + width] \
                    .rearrange("(k p) c -> p k c", p=128)
                src = zt[:, 0:width].rearrange("p (k c) -> p k c", k=1) \
                    .broadcast_to([128, k, width])
                return engine.dma_start(out=dst, in_=src)

            # Ring balance (writes): SP carries bands+elem (7.0 MB), ACT
            # carries kcl+kvl (6.5 MB).  SWDGE is deliberately unused for
            # bulk data — a third queue on the shared SDMA pool lowers the
            # aggregate rate (measured).  Both rings end on large-descriptor
            # zero fills; the small-descriptor bands DMA sits mid-queue on
            # SP where its lower drain rate overlaps other traffic.
            zfill(nc.scalar, 0, RK, E, W - E)         # kcl rows, cols E:W
            zfill(nc.scalar, RK, RE, 0, 2 * E)        # kvl rows, cols 0:2E

            # Six [128, 256] half-bands via affine_select: keep in_[p, c]
            # where c - p - 128k == 0, fill 0.  Result is [diag|0] (k=0) or
            # [0|diag] (k=1).
            # Each input is a [128, 1] value column broadcast along the free
            # dim with a step-0 AP (no materialized broadcast tile needed).
            def bc(col):
                return col.broadcast_to([128, RE])

            bt = pool.tile([128, 6 * RE], f32, tag="bt")
            srcs = [ones[:, 0:1], ones[:, 0:1],
                    vbt[:, 0:1], vbt[:, 1:2],
                    vbt[:, 2:3], vbt[:, 3:4]]
            for j, src in enumerate(srcs):
                nc.gpsimd.affine_select(
                    bt[:, RE * j:RE * (j + 1)], bc(src),
                    pattern=[[1, RE]],
                    compare_op=mybir.AluOpType.is_equal,
                    fill=0.0, base=-128 * (j % 2), channel_multiplier=-1,
                )

            # One DMA for all six half-bands, SBUF layout preserved (6 KB
            # descriptors), followed by the big elem zero fill so the SP
            # ring's tail is a fast large-descriptor transfer.  The queued
            # vbt+mrow data keeps SP busy while bands waits on the selects.
            bands_dma = nc.sync.dma_start(out=out_bands[:, :], in_=bt[:, :])
            add_dep_helper(bands_dma.ins, mrow_dma.ins, sync=False,
                           reason="bands third in the SP FIFO")
            elem_dma = zfill(nc.sync, RK + RE, RE, 0, W)  # elem rows, full width
            add_dep_helper(elem_dma.ins, bands_dma.ins, sync=False,
                           reason="elem fill in the SP tail")

    _split_waits(nc)
    return nc


def _split_waits(nc, maxw=1):
    """This walrus build rejects instructions carrying more than one
    sync-wait ("Too many sync wait commands").  Tile can emit several on one
    instruction (notably the kernel-tail Drain).  Hoist the extras onto
    same-engine NoOps inserted immediately before the instruction."""
    import concourse.mybir as mybir

    nsplit = 0
    for fn in nc.m.functions:
        for blk in fn.blocks:
            newlist = []
            changed = False
            for inst in blk.instructions:
                si = inst.sync_info
                ow = list(si.on_wait) if si is not None and si.on_wait else []
                if len(ow) > maxw:
                    head, tail = ow[:-maxw], ow[-maxw:]
                    for w in head:
                        nop = mybir.InstNoOp(name=f"nopw-{nsplit}", ins=[], outs=[])
                        nsplit += 1
                        nop.engine = inst.engine
                        nop.sync_info = mybir.SyncInfo(on_wait=[w], on_update=[])
                        newlist.append(nop)
                    inst.sync_info = mybir.SyncInfo(
                        on_wait=tail,
                        on_update=list(si.on_update) if si.on_update else [])
                    changed = True
                newlist.append(inst)
            if changed:
                blk.instructions = newlist
    return nsplit


def _element_vals(params, sw_params, kinds, time):
    """Host replica of reference._element_vals (numpy, f32)."""
    params = np.asarray(params, dtype=np.float32)
    sw_params = np.asarray(sw_params, dtype=np.float32)
    kinds = np.asarray(kinds)
    t = int(time)
    sw_on = sw_params[:, t] > 0  # sigmoid(x) > 0.5  <=>  x > 0
    one = np.ones_like(params)
    zero = np.zeros_like(params)
    ndt = (np.float32(-DT) / params).astype(np.float32)
    z_vals = np.select(
        [kinds == 0, kinds == 1, kinds == 2, kinds == 3, kinds == 4, kinds == 5],
        [-params, zero, one, np.where(sw_on, 0.0, 1.0).astype(np.float32), ndt, one],
    ).astype(np.float32)
    y_vals = np.select(
        [kinds == 0, kinds == 1, kinds == 2, kinds == 3, kinds == 4, kinds == 5],
        [one, one, zero, np.where(sw_on, 1.0, 0.0).astype(np.float32), one, ndt],
    ).astype(np.float32)
    return z_vals, y_vals


def _run(M, params, sw_params, kinds, time, trace=False):
    from concourse.bass_utils import run_bass_kernel_spmd

    M = np.ascontiguousarray(np.asarray(M, dtype=np.float32))
    z_vals, y_vals = _element_vals(params, sw_params, kinds, time)
    negMt = -(M.T)  # [E, N] C-contiguous

    in_maps = []
    for c in range(C):
        # [128, 4] value columns (z0, z1, y0, y1): col k holds
        # vals[128*(k%2) + p] for this core's 256-element slice.
        zc = z_vals[RE * c:RE * (c + 1)].reshape(2, 128).T
        yc = y_vals[RE * c:RE * (c + 1)].reshape(2, 128).T
        in_maps.append({
            "mrow": M[RK * c:RK * (c + 1), :],
            "negmt": negMt[RE * c:RE * (c + 1), :],
            "vb": np.ascontiguousarray(np.concatenate([zc, yc], axis=1)),
        })

    if "nc" not in _cache:
        _cache["nc"] = _build_nc()
    res = run_bass_kernel_spmd(
        _cache["nc"], in_maps, core_ids=list(range(C)), trace=trace,
        trace_cores=list(range(C)) if trace else None,
    )

    full = np.empty((N + 2 * E, 2 * E + N), dtype=np.float32)
    for c in range(C):
        r = res.results[c]
        om = r["out_main"]
        full[RK * c:RK * (c + 1), :] = om[0:RK]
        full[N + RE * c:N + RE * (c + 1), :] = om[RK:RK + RE]
        full[N + E + RE * c:N + E + RE * (c + 1), :] = om[RK + RE:RK + 2 * RE]
        # overlay core-dependent diagonal bands; out_bands is [128, 6*256]
        # with half-bands (i0 i1 z0 z1 y0 y1) along the free dim
        bands = r["out_bands"].reshape(128, 6, RE).transpose(1, 0, 2)
        ib = bands[0:2].reshape(RE, RE)
        zb = bands[2:4].reshape(RE, RE)
        yb = bands[4:6].reshape(RE, RE)
        full[N + RE * c:N + RE * (c + 1), E + RE * c:E + RE * (c + 1)] = ib
        full[N + E + RE * c:N + E + RE * (c + 1), RE * c:RE * (c + 1)] = zb
        full[N + E + RE * c:N + E + RE * (c + 1), E + RE * c:E + RE * (c + 1)] = yb
    return full, res


def kernel(M, params, sw_params, kinds, time):
    out, _ = _run(M, params, sw_params, kinds, time, trace=False)
    return out



# revision 2
# speedup vs baseline: 2.4897x; 2.4897x over previous
"""Trainium2 Bass kernel for nn_Coefficients: assemble the MNA coefficient
block matrix  [[M, 0, 0], [0, I, -M^T], [diag(z), diag(y), 0]]  of shape
[N+2E, 2E+N] from M [N,E], params/kinds/sw_params.

Sharding (8 cores, SPMD — one program, per-core data):
  core c owns kcl rows [128c,128c+128), kvl rows e in [256c,256c+256) and
  elem rows e in the same range.  Each core's out_main [640, 5120] holds its
  kcl/kvl/elem row chunks; out_bands [128, 6*256] holds the three 256x256
  diagonal blocks (identity, diag(z), diag(y)) packed as six 128x256
  half-bands, whose global column position depends on the core; the host
  unshard step places rows and overlays bands into the full [5120, 5120]
  output.

The PJRT execution path donates zero-initialised buffers as the kernel's
ExternalOutputs (see bass2jax.run_bass_via_pjrt zero_outs/donate_argnums —
kernels that don't write every element rely on that, and
test_bass2jax.py::test_donation guards it).  The structural-zero regions of
out_main therefore need no DMA traffic at all: the device writes only the
data-dependent bytes — the M row block, the -M^T block and the diagonal
bands — cutting per-core HBM traffic from ~15.9 MB to ~4.8 MB.

The toolchain allows only one sync-wait per instruction, so extra waits are
hoisted onto same-engine NoOps (_split_waits).
"""

import numpy as np

N, E, SIG = 1024, 2048, 64
C = 8            # cores
RK = N // C      # 128 kcl rows per core
RE = E // C      # 256 kvl/elem rows per core
W = 2 * E + N    # 5120 output width
DT = 1e-6

_cache = {}


def _build_nc():
    import concourse.bass as bass
    import concourse.mybir as mybir
    from concourse.tile import TileContext

    f32 = mybir.dt.float32
    nc = bass.Bass(name="coeffs_scatter", enable_partition_id=False)

    mrow = nc.dram_tensor("mrow", [RK, E], f32, kind="ExternalInput")
    negmt = nc.dram_tensor("negmt", [RE, N], f32, kind="ExternalInput")
    # Diagonal values [128, 5]: cols (z0, z1, y0, y1, ones); col k holds
    # vals[128*(k%2) + p] at row p for k<4, col 4 is all-ones (the identity
    # band source).  Broadcast on-chip via step-0 APs.
    vb = nc.dram_tensor("vb", [128, 5], f32, kind="ExternalInput")

    out_main = nc.dram_tensor("out_main", [RK + 2 * RE, W], f32, kind="ExternalOutput")
    # Six [128, 256] half-bands (i0 i1 z0 z1 y0 y1) packed along the free
    # dim — SBUF layout dumped verbatim so the DMA gets 6 KB descriptors;
    # the host unpacks.
    out_bands = nc.dram_tensor("out_bands", [128, 6 * RE], f32, kind="ExternalOutput")

    with TileContext(nc) as tc:
        with tc.tile_pool(name="pool", bufs=1) as pool:
            # Band value load first on the SP ring; its consumers are the
            # affine_selects feeding the band DMA.
            vbt = pool.tile([128, 5], f32, tag="vbt")
            nc.sync.dma_start(out=vbt[:], in_=vb[:, :])

            # Data blocks: M rows into the kcl block, -M^T rows into the
            # kvl right block.  DRAM->DRAM on the ACT ring, no deps.
            nc.scalar.dma_start(out=out_main[0:RK, 0:E], in_=mrow[:, :])
            nc.scalar.dma_start(out=out_main[RK:RK + RE, 2 * E:W], in_=negmt[:, :])

            # Six [128, 256] half-bands via affine_select: keep in_[p, c]
            # where c - p - 128k == 0, fill 0.  Result is [diag|0] (k=0) or
            # [0|diag] (k=1).  Each input is a [128, 1] value column
            # broadcast along the free dim with a step-0 AP.
            bt = pool.tile([128, 6 * RE], f32, tag="bt")
            srcs = [vbt[:, 4:5], vbt[:, 4:5],
                    vbt[:, 0:1], vbt[:, 1:2],
                    vbt[:, 2:3], vbt[:, 3:4]]
            for j, src in enumerate(srcs):
                nc.gpsimd.affine_select(
                    bt[:, RE * j:RE * (j + 1)], src.broadcast_to([128, RE]),
                    pattern=[[1, RE]],
                    compare_op=mybir.AluOpType.is_equal,
                    fill=0.0, base=-128 * (j % 2), channel_multiplier=-1,
                )

            # One DMA for all six half-bands, SBUF layout preserved (6 KB
            # descriptors) on the SP ring.
            nc.sync.dma_start(out=out_bands[:, :], in_=bt[:, :])

    _strip_const_memsets(nc)
    _split_waits(nc)
    return nc


def _strip_const_memsets(nc):
    """Drop the framework's const-AP memsets (const-f32-0.0 etc.) from the
    entry block.  Nothing in this kernel reads those SBUF tiles, and they
    carry no sync info, so removing the writes cannot change any output."""
    import concourse.mybir as mybir

    for fn in nc.m.functions:
        for blk in fn.blocks:
            keep = []
            for inst in blk.instructions:
                if isinstance(inst, mybir.InstMemset):
                    outs = getattr(inst, "outs", [])
                    names = [getattr(o, "name", "") for o in outs]
                    si = inst.sync_info
                    no_sync = si is None or (not si.on_wait and not si.on_update)
                    if no_sync and names and all(n.startswith("const-") for n in names):
                        continue
                keep.append(inst)
            blk.instructions = keep


def _split_waits(nc, maxw=1):
    """This walrus build rejects instructions carrying more than one
    sync-wait ("Too many sync wait commands").  Tile can emit several on one
    instruction (notably the kernel-tail Drain).  Hoist the extras onto
    same-engine NoOps inserted immediately before the instruction."""
    import concourse.mybir as mybir

    nsplit = 0
    for fn in nc.m.functions:
        for blk in fn.blocks:
            newlist = []
            changed = False
            for inst in blk.instructions:
                si = inst.sync_info
                ow = list(si.on_wait) if si is not None and si.on_wait else []
                if len(ow) > maxw:
                    head, tail = ow[:-maxw], ow[-maxw:]
                    for w in head:
                        nop = mybir.InstNoOp(name=f"nopw-{nsplit}", ins=[], outs=[])
                        nsplit += 1
                        nop.engine = inst.engine
                        nop.sync_info = mybir.SyncInfo(on_wait=[w], on_update=[])
                        newlist.append(nop)
                    inst.sync_info = mybir.SyncInfo(
                        on_wait=tail,
                        on_update=list(si.on_update) if si.on_update else [])
                    changed = True
                newlist.append(inst)
            if changed:
                blk.instructions = newlist
    return nsplit


def _element_vals(params, sw_params, kinds, time):
    """Host replica of reference._element_vals (numpy, f32)."""
    params = np.asarray(params, dtype=np.float32)
    sw_params = np.asarray(sw_params, dtype=np.float32)
    kinds = np.asarray(kinds)
    t = int(time)
    sw_on = sw_params[:, t] > 0  # sigmoid(x) > 0.5  <=>  x > 0
    one = np.ones_like(params)
    zero = np.zeros_like(params)
    ndt = (np.float32(-DT) / params).astype(np.float32)
    z_vals = np.select(
        [kinds == 0, kinds == 1, kinds == 2, kinds == 3, kinds == 4, kinds == 5],
        [-params, zero, one, np.where(sw_on, 0.0, 1.0).astype(np.float32), ndt, one],
    ).astype(np.float32)
    y_vals = np.select(
        [kinds == 0, kinds == 1, kinds == 2, kinds == 3, kinds == 4, kinds == 5],
        [one, one, zero, np.where(sw_on, 1.0, 0.0).astype(np.float32), one, ndt],
    ).astype(np.float32)
    return z_vals, y_vals


def _run(M, params, sw_params, kinds, time, trace=False):
    from concourse.bass_utils import run_bass_kernel_spmd

    M = np.ascontiguousarray(np.asarray(M, dtype=np.float32))
    z_vals, y_vals = _element_vals(params, sw_params, kinds, time)
    negMt = -(M.T)  # [E, N] C-contiguous

    ones = np.ones((128, 1), dtype=np.float32)
    in_maps = []
    for c in range(C):
        # [128, 5] value columns (z0, z1, y0, y1, ones): col k holds
        # vals[128*(k%2) + p] for this core's 256-element slice.
        zc = z_vals[RE * c:RE * (c + 1)].reshape(2, 128).T
        yc = y_vals[RE * c:RE * (c + 1)].reshape(2, 128).T
        in_maps.append({
            "mrow": M[RK * c:RK * (c + 1), :],
            "negmt": negMt[RE * c:RE * (c + 1), :],
            "vb": np.ascontiguousarray(np.concatenate([zc, yc, ones], axis=1)),
        })

    if "nc" not in _cache:
        _cache["nc"] = _build_nc()
    res = run_bass_kernel_spmd(
        _cache["nc"], in_maps, core_ids=list(range(C)), trace=trace,
        trace_cores=list(range(C)) if trace else None,
    )

    full = np.empty((N + 2 * E, 2 * E + N), dtype=np.float32)
    for c in range(C):
        r = res.results[c]
        om = r["out_main"]
        full[RK * c:RK * (c + 1), :] = om[0:RK]
        full[N + RE * c:N + RE * (c + 1), :] = om[RK:RK + RE]
        full[N + E + RE * c:N + E + RE * (c + 1), :] = om[RK + RE:RK + 2 * RE]
        # overlay core-dependent diagonal bands; out_bands is [128, 6*256]
        # with half-bands (i0 i1 z0 z1 y0 y1) along the free dim
        bands = r["out_bands"].reshape(128, 6, RE).transpose(1, 0, 2)
        ib = bands[0:2].reshape(RE, RE)
        zb = bands[2:4].reshape(RE, RE)
        yb = bands[4:6].reshape(RE, RE)
        full[N + RE * c:N + RE * (c + 1), E + RE * c:E + RE * (c + 1)] = ib
        full[N + E + RE * c:N + E + RE * (c + 1), RE * c:RE * (c + 1)] = zb
        full[N + E + RE * c:N + E + RE * (c + 1), E + RE * c:E + RE * (c + 1)] = yb
    return full, res


def kernel(M, params, sw_params, kinds, time):
    out, _ = _run(M, params, sw_params, kinds, time, trace=False)
    return out


# revision 4
# speedup vs baseline: 2.8376x; 1.1397x over previous
"""Trainium2 Bass kernel for nn_Coefficients: assemble the MNA coefficient
block matrix  [[M, 0, 0], [0, I, -M^T], [diag(z), diag(y), 0]]  of shape
[N+2E, 2E+N] from M [N,E], params/kinds/sw_params.

Sharding (8 cores, SPMD — one program, per-core data):
  core c owns kcl rows [128c,128c+128), kvl rows e in [256c,256c+256) and
  elem rows e in the same range.  Each core's out_main [640, 5120] holds its
  kcl/kvl/elem row chunks; out_bands [128, 6*256] holds the three 256x256
  diagonal blocks (identity, diag(z), diag(y)) packed as six 128x256
  half-bands, whose global column position depends on the core; the host
  unshard step places rows and overlays bands into the full [5120, 5120]
  output.

The PJRT execution path donates zero-initialised buffers as the kernel's
ExternalOutputs (see bass2jax.run_bass_via_pjrt zero_outs/donate_argnums —
kernels that don't write every element rely on that, and
test_bass2jax.py::test_donation guards it).  The structural-zero regions of
out_main therefore need no DMA traffic at all: the device writes only the
data-dependent bytes — the M row block, the -M^T block and the diagonal
bands — cutting per-core HBM traffic from ~15.9 MB to ~4.8 MB.

The toolchain allows only one sync-wait per instruction, so extra waits are
hoisted onto same-engine NoOps (_split_waits).
"""

import numpy as np

N, E, SIG = 1024, 2048, 64
C = 8            # cores
RK = N // C      # 128 kcl rows per core
RE = E // C      # 256 kvl/elem rows per core
W = 2 * E + N    # 5120 output width
DT = 1e-6

_cache = {}


def _build_nc():
    import concourse.bass as bass
    import concourse.mybir as mybir
    from concourse.tile import TileContext

    f32 = mybir.dt.float32
    nc = bass.Bass(name="coeffs_scatter", enable_partition_id=False)

    mrow = nc.dram_tensor("mrow", [RK, E], f32, kind="ExternalInput")
    negmt = nc.dram_tensor("negmt", [RE, N], f32, kind="ExternalInput")
    # Diagonal values [128, 5]: cols (z0, z1, y0, y1, ones); col k holds
    # vals[128*(k%2) + p] at row p for k<4, col 4 is all-ones (the identity
    # band source).  Broadcast on-chip via step-0 APs.
    vb = nc.dram_tensor("vb", [128, 5], f32, kind="ExternalInput")

    out_main = nc.dram_tensor("out_main", [RK + 2 * RE, W], f32, kind="ExternalOutput")
    # Six [128, 256] half-bands (i0 i1 z0 z1 y0 y1) packed along the free
    # dim — SBUF layout dumped verbatim so the DMA gets 6 KB descriptors;
    # the host unpacks.
    out_bands = nc.dram_tensor("out_bands", [128, 6 * RE], f32, kind="ExternalOutput")

    with TileContext(nc) as tc:
        with tc.tile_pool(name="pool", bufs=1) as pool:
            # Band value load first on the SP ring; its consumers are the
            # affine_selects feeding the band DMA.
            vbt = pool.tile([128, 5], f32, tag="vbt")
            nc.sync.dma_start(out=vbt[:], in_=vb[:, :])

            # Data blocks: M rows into the kcl block, -M^T rows into the
            # kvl right block.  DRAM->DRAM on the ACT ring, no deps.
            nc.scalar.dma_start(out=out_main[0:RK, 0:E], in_=mrow[:, :])
            nc.scalar.dma_start(out=out_main[RK:RK + RE, 2 * E:W], in_=negmt[:, :])

            # Six [128, 256] half-bands via affine_select: keep in_[p, c]
            # where c - p - 128k == 0, fill 0.  Result is [diag|0] (k=0) or
            # [0|diag] (k=1).  Each input is a [128, 1] value column
            # broadcast along the free dim with a step-0 AP.
            bt = pool.tile([128, 6 * RE], f32, tag="bt")
            srcs = [vbt[:, 4:5], vbt[:, 4:5],
                    vbt[:, 0:1], vbt[:, 1:2],
                    vbt[:, 2:3], vbt[:, 3:4]]
            for j, src in enumerate(srcs):
                nc.gpsimd.affine_select(
                    bt[:, RE * j:RE * (j + 1)], src.broadcast_to([128, RE]),
                    pattern=[[1, RE]],
                    compare_op=mybir.AluOpType.is_equal,
                    fill=0.0, base=-128 * (j % 2), channel_multiplier=-1,
                )

            # One DMA for all six half-bands, SBUF layout preserved (6 KB
            # descriptors) on the SP ring.
            nc.sync.dma_start(out=out_bands[:, :], in_=bt[:, :])

    _strip_const_memsets(nc)
    _trim_end_barriers(nc)
    _split_waits(nc)
    return nc


def _trim_end_barriers(nc):
    """Restructure the kernel-end block so only DVE waits for DMA
    completion and only Pool runs the tile-sem RANGE_CLEAR behind a single
    DVE->Pool handshake; PE/Act/SP return immediately.

    Rationale: on NEFF return the runtime appends a per-engine semaphore
    reset train (~51 clears each, observed fixed mapping PE->S[2..53],
    Act->S[54..104], Pool->S[105..155], DVE->S[156..206], SP->S[207..255]).
    With the stock double all-engine barrier, every train runs after the
    last DMA lands, and PE's train (~6 us, slowest dispatch) sits on the
    critical path.  The only semaphores live at kernel end are the DMAHW
    sems 156-163 (incremented by in-flight DMA completions; in DVE's
    range) and the tile/barrier sems 151-163 cleared by Pool's RANGE_CLEAR
    and train.  PE/Act/SP's ranges are dead by the time their trains can
    start, so those engines may return while DMAs drain — their trains are
    hidden under the DMA window — provided DVE still waits for all DMA
    sems before returning and Pool's RANGE_CLEAR stays behind DVE's
    confirmation (the gather handshake)."""
    import concourse.mybir as mybir

    ET = mybir.EngineType
    end_blk = None
    for fn in nc.m.functions:
        for blk in fn.blocks:
            if blk.name.endswith("_end"):
                end_blk = blk
    assert end_blk is not None

    def waits(inst):
        si = inst.sync_info
        return list(si.on_wait) if si is not None and si.on_wait else []

    # Harvest the DMA-completion waits currently parked on SP.
    dma_waits = []
    for inst in end_blk.instructions:
        if inst.engine == ET.SP and isinstance(inst, (mybir.InstNoOp, mybir.InstDrain)):
            for w in waits(inst):
                if "DMAHW" in (w.ant_name or ""):
                    dma_waits.append(w)
    assert len(dma_waits) == 4, [w.ant_name for w in dma_waits]

    seen = {ET.SP: 0, ET.Activation: 0, ET.PE: 0, ET.DVE: 0, ET.Pool: 0}
    out = []
    for inst in end_blk.instructions:
        eng = inst.engine
        if eng == ET.SP:
            if isinstance(inst, mybir.InstNoOp):
                continue  # waits harvested above
            if isinstance(inst, mybir.InstDrain) and seen[eng] == 0:
                seen[eng] += 1
                inst.sync_info = mybir.SyncInfo(on_wait=[], on_update=[])
                out.append(inst)  # plain pipeline flush, no waits
                continue
            continue  # remaining SP barrier insts dropped
        if eng == ET.Activation:
            if isinstance(inst, mybir.InstDrain) and seen[eng] == 0:
                seen[eng] += 1
                inst.sync_info = mybir.SyncInfo(on_wait=[], on_update=[])
                out.append(inst)
                continue
            continue
        if eng == ET.PE:
            continue  # no work, no barrier: return immediately
        if eng == ET.DVE:
            if isinstance(inst, mybir.InstDrain) and seen[eng] == 0:
                seen[eng] += 1
                # NoOps carrying all but the last DMA wait, then the Drain
                # waits the last one and signals Pool via the gather inc.
                for i, w in enumerate(dma_waits[:-1]):
                    nop = mybir.InstNoOp(name=f"dvew-{i}", ins=[], outs=[])
                    nop.engine = ET.DVE
                    nop.sync_info = mybir.SyncInfo(on_wait=[w], on_update=[])
                    out.append(nop)
                si = inst.sync_info
                ups = list(si.on_update) if si is not None and si.on_update else []
                assert any("gather" in (u.ant_name or "") for u in ups)
                inst.sync_info = mybir.SyncInfo(
                    on_wait=[dma_waits[-1]], on_update=ups)
                out.append(inst)
                continue
            continue  # barrier EventSemaphores + second round dropped
        if eng == ET.Pool:
            if isinstance(inst, mybir.InstEventSemaphore):
                si = inst.sync_info
                ws = waits(inst)
                if seen[eng] == 0 and ws and "gather" in (ws[0].ant_name or ""):
                    seen[eng] += 1
                    # only DVE increments now: wait 4 -> 1, sub 4 -> 1
                    ws[0].wait_value = 1
                    ups = list(si.on_update) if si and si.on_update else []
                    for u in ups:
                        if "gather" in (u.ant_name or ""):
                            u.update_value = 1
                    inst.sync_info = mybir.SyncInfo(on_wait=ws, on_update=ups)
                    out.append(inst)
                    continue
                continue  # release EventSemaphores + second round dropped
            if isinstance(inst, mybir.InstDrain):
                if seen[eng] >= 1 or not out or True:
                    # keep Pool drains up to the ISA range-clear; drop the
                    # trailing second-round drain (appears after the ISA)
                    if any(isinstance(x, mybir.InstISA) for x in out
                           if x.engine == ET.Pool):
                        continue
                    out.append(inst)
                    continue
            out.append(inst)  # the InstISA range-clear
            continue
        out.append(inst)
    end_blk.instructions = out


def _strip_const_memsets(nc):
    """Drop the framework's const-AP memsets (const-f32-0.0 etc.) from the
    entry block.  Nothing in this kernel reads those SBUF tiles, and they
    carry no sync info, so removing the writes cannot change any output."""
    import concourse.mybir as mybir

    for fn in nc.m.functions:
        for blk in fn.blocks:
            keep = []
            for inst in blk.instructions:
                if isinstance(inst, mybir.InstMemset):
                    outs = getattr(inst, "outs", [])
                    names = [getattr(o, "memref", "") or "" for o in outs]
                    si = inst.sync_info
                    no_sync = si is None or (not si.on_wait and not si.on_update)
                    if no_sync and names and all(n.startswith("const-") for n in names):
                        continue
                keep.append(inst)
            blk.instructions = keep


def _split_waits(nc, maxw=1):
    """This walrus build rejects instructions carrying more than one
    sync-wait ("Too many sync wait commands").  Tile can emit several on one
    instruction (notably the kernel-tail Drain).  Hoist the extras onto
    same-engine NoOps inserted immediately before the instruction."""
    import concourse.mybir as mybir

    nsplit = 0
    for fn in nc.m.functions:
        for blk in fn.blocks:
            newlist = []
            changed = False
            for inst in blk.instructions:
                si = inst.sync_info
                ow = list(si.on_wait) if si is not None and si.on_wait else []
                if len(ow) > maxw:
                    head, tail = ow[:-maxw], ow[-maxw:]
                    for w in head:
                        nop = mybir.InstNoOp(name=f"nopw-{nsplit}", ins=[], outs=[])
                        nsplit += 1
                        nop.engine = inst.engine
                        nop.sync_info = mybir.SyncInfo(on_wait=[w], on_update=[])
                        newlist.append(nop)
                    inst.sync_info = mybir.SyncInfo(
                        on_wait=tail,
                        on_update=list(si.on_update) if si.on_update else [])
                    changed = True
                newlist.append(inst)
            if changed:
                blk.instructions = newlist
    return nsplit


def _element_vals(params, sw_params, kinds, time):
    """Host replica of reference._element_vals (numpy, f32)."""
    params = np.asarray(params, dtype=np.float32)
    sw_params = np.asarray(sw_params, dtype=np.float32)
    kinds = np.asarray(kinds)
    t = int(time)
    sw_on = sw_params[:, t] > 0  # sigmoid(x) > 0.5  <=>  x > 0
    one = np.ones_like(params)
    zero = np.zeros_like(params)
    ndt = (np.float32(-DT) / params).astype(np.float32)
    z_vals = np.select(
        [kinds == 0, kinds == 1, kinds == 2, kinds == 3, kinds == 4, kinds == 5],
        [-params, zero, one, np.where(sw_on, 0.0, 1.0).astype(np.float32), ndt, one],
    ).astype(np.float32)
    y_vals = np.select(
        [kinds == 0, kinds == 1, kinds == 2, kinds == 3, kinds == 4, kinds == 5],
        [one, one, zero, np.where(sw_on, 1.0, 0.0).astype(np.float32), one, ndt],
    ).astype(np.float32)
    return z_vals, y_vals


def _run(M, params, sw_params, kinds, time, trace=False):
    from concourse.bass_utils import run_bass_kernel_spmd

    M = np.ascontiguousarray(np.asarray(M, dtype=np.float32))
    z_vals, y_vals = _element_vals(params, sw_params, kinds, time)
    negMt = -(M.T)  # [E, N] C-contiguous

    ones = np.ones((128, 1), dtype=np.float32)
    in_maps = []
    for c in range(C):
        # [128, 5] value columns (z0, z1, y0, y1, ones): col k holds
        # vals[128*(k%2) + p] for this core's 256-element slice.
        zc = z_vals[RE * c:RE * (c + 1)].reshape(2, 128).T
        yc = y_vals[RE * c:RE * (c + 1)].reshape(2, 128).T
        in_maps.append({
            "mrow": M[RK * c:RK * (c + 1), :],
            "negmt": negMt[RE * c:RE * (c + 1), :],
            "vb": np.ascontiguousarray(np.concatenate([zc, yc, ones], axis=1)),
        })

    if "nc" not in _cache:
        _cache["nc"] = _build_nc()
    res = run_bass_kernel_spmd(
        _cache["nc"], in_maps, core_ids=list(range(C)), trace=trace,
        trace_cores=list(range(C)) if trace else None,
    )

    full = np.empty((N + 2 * E, 2 * E + N), dtype=np.float32)
    for c in range(C):
        r = res.results[c]
        om = r["out_main"]
        full[RK * c:RK * (c + 1), :] = om[0:RK]
        full[N + RE * c:N + RE * (c + 1), :] = om[RK:RK + RE]
        full[N + E + RE * c:N + E + RE * (c + 1), :] = om[RK + RE:RK + 2 * RE]
        # overlay core-dependent diagonal bands; out_bands is [128, 6*256]
        # with half-bands (i0 i1 z0 z1 y0 y1) along the free dim
        bands = r["out_bands"].reshape(128, 6, RE).transpose(1, 0, 2)
        ib = bands[0:2].reshape(RE, RE)
        zb = bands[2:4].reshape(RE, RE)
        yb = bands[4:6].reshape(RE, RE)
        full[N + RE * c:N + RE * (c + 1), E + RE * c:E + RE * (c + 1)] = ib
        full[N + E + RE * c:N + E + RE * (c + 1), RE * c:RE * (c + 1)] = zb
        full[N + E + RE * c:N + E + RE * (c + 1), E + RE * c:E + RE * (c + 1)] = yb
    return full, res


def kernel(M, params, sw_params, kinds, time):
    out, _ = _run(M, params, sw_params, kinds, time, trace=False)
    return out


# revision 10
# speedup vs baseline: 3.0181x; 1.0636x over previous
"""Trainium2 Bass kernel for nn_Coefficients: assemble the MNA coefficient
block matrix  [[M, 0, 0], [0, I, -M^T], [diag(z), diag(y), 0]]  of shape
[N+2E, 2E+N] from M [N,E], params/kinds/sw_params.

Sharding (8 cores, SPMD — one program, per-core data):
  core c owns kcl rows [128c,128c+128), kvl rows e in [256c,256c+256) and
  elem rows e in the same range.  Each core's out_main [640, 5120] holds its
  kcl/kvl/elem row chunks; out_bands [128, 6*256] holds the three 256x256
  diagonal blocks (identity, diag(z), diag(y)) packed as six 128x256
  half-bands, whose global column position depends on the core; the host
  unshard step places rows and overlays bands into the full [5120, 5120]
  output.

The PJRT execution path donates zero-initialised buffers as the kernel's
ExternalOutputs (see bass2jax.run_bass_via_pjrt zero_outs/donate_argnums —
kernels that don't write every element rely on that, and
test_bass2jax.py::test_donation guards it).  The structural-zero regions of
out_main therefore need no DMA traffic at all: the device writes only the
data-dependent bytes — the M row block, the -M^T block and the diagonal
bands — cutting per-core HBM traffic from ~15.9 MB to ~4.8 MB.

The toolchain allows only one sync-wait per instruction, so extra waits are
hoisted onto same-engine NoOps (_split_waits).
"""

import numpy as np

N, E, SIG = 1024, 2048, 64
C = 8            # cores
RK = N // C      # 128 kcl rows per core
RE = E // C      # 256 kvl/elem rows per core
W = 2 * E + N    # 5120 output width
DT = 1e-6

_cache = {}


def _build_nc():
    import concourse.bass as bass
    import concourse.mybir as mybir
    from concourse.tile import TileContext

    f32 = mybir.dt.float32
    nc = bass.Bass(name="coeffs_scatter", enable_partition_id=False)

    mrow = nc.dram_tensor("mrow", [RK, E], f32, kind="ExternalInput")
    negmt = nc.dram_tensor("negmt", [RE, N], f32, kind="ExternalInput")
    # Diagonal values [128, 4]: cols (z0, z1, y0, y1); col k holds
    # vals[128*(k%2) + p] at row p.
    vb = nc.dram_tensor("vb", [128, 4], f32, kind="ExternalInput")
    # Constant diagonal masks [128, 2*RE]: cols 0:RE = eye(128,RE,0)
    # ([diag|0]), cols RE:2*RE = eye(128,RE,128) ([0|diag]).  Serves as the
    # identity band content directly and as the multiplicand for the z/y
    # bands (the preloaded-constant idiom, like the PE-transpose identity).
    mask = nc.dram_tensor("mask", [128, 2 * RE], f32, kind="ExternalInput")

    out_main = nc.dram_tensor("out_main", [RK + 2 * RE, W], f32, kind="ExternalOutput")
    # Six [128, 256] half-bands (i0 i1 z0 z1 y0 y1) packed along the free
    # dim — SBUF layout dumped verbatim so the DMA gets 6 KB descriptors;
    # the host unpacks.
    out_bands = nc.dram_tensor("out_bands", [128, 6 * RE], f32, kind="ExternalOutput")

    with TileContext(nc) as tc:
        with tc.tile_pool(name="pool", bufs=1) as pool:
            # Loads on the SP ring: band values + diagonal masks.  (Hoisted
            # to the main block pre-barrier by _hoist_dmas_to_main.)
            vbt = pool.tile([128, 4], f32, tag="vbt")
            nc.sync.dma_start(out=vbt[:], in_=vb[:, :])
            mt = pool.tile([128, 2 * RE], f32, tag="mt")
            nc.sync.dma_start(out=mt[:], in_=mask[:, :])

            # ACT ring, no deps, off-peak: M rows into the kcl block, -M^T
            # rows into the kvl right block, and the identity bands (i0 i1),
            # which are exactly the mask content, DRAM->DRAM.
            nc.scalar.dma_start(out=out_main[0:RK, 0:E], in_=mrow[:, :])
            nc.scalar.dma_start(out=out_main[RK:RK + RE, 2 * E:W], in_=negmt[:, :])
            nc.scalar.dma_start(out=out_bands[:, 0:2 * RE], in_=mask[:, :])

            # z/y half-bands: mask half times per-partition value column.
            # Two multiplies per engine (Pool: z, DVE: y), in parallel.
            bt = pool.tile([128, 4 * RE], f32, tag="bt")
            for j in range(4):
                eng = nc.gpsimd if j < 2 else nc.vector
                eng.tensor_scalar_mul(
                    bt[:, RE * j:RE * (j + 1)],
                    mt[:, RE * (j % 2):RE * (j % 2 + 1)],
                    vbt[:, j:j + 1],
                )

            # One DMA for the four computed half-bands (z0 z1 y0 y1), SBUF
            # layout preserved, on the otherwise-idle SP ring.
            nc.sync.dma_start(out=out_bands[:, 2 * RE:], in_=bt[:, :])

    _strip_const_memsets(nc)
    _hoist_dmas_to_main(nc)
    _trim_end_barriers(nc)
    _split_waits(nc)
    return nc


def _hoist_dmas_to_main(nc):
    """Move the dependency-free DMA dispatches (vbt load, mrow, negmt) from
    the tile block into the entry block, before the head all-engine
    barrier, so their descriptors are generated ~0.7 us earlier and the
    bulk DRAM->DRAM traffic drains before the band chain needs HBM."""
    import concourse.mybir as mybir

    main_blk = None
    tile_blk = None
    for fn in nc.m.functions:
        for blk in fn.blocks:
            if blk.name == "main":
                main_blk = blk
            elif not blk.name.endswith("_end") and blk.name != "main":
                tile_blk = blk
    assert main_blk is not None and tile_blk is not None

    def waits(inst):
        si = inst.sync_info
        return list(si.on_wait) if si is not None and si.on_wait else []

    # dependency-free DMA copies only (no on_wait)
    hoist = [i for i in tile_blk.instructions
             if isinstance(i, mybir.InstDMACopy) and not waits(i)]
    tile_blk.instructions = [i for i in tile_blk.instructions if i not in hoist]

    # insert each before its engine's first Drain (the head barrier)
    out = []
    placed = set()
    for inst in main_blk.instructions:
        if isinstance(inst, mybir.InstDrain):
            for h in hoist:
                if h.engine == inst.engine and id(h) not in placed:
                    out.append(h)
                    placed.add(id(h))
        out.append(inst)
    assert len(placed) == len(hoist), (len(placed), len(hoist))
    main_blk.instructions = out


def _trim_end_barriers(nc):
    """Restructure the kernel-end block so only DVE waits for DMA
    completion and only Pool runs the tile-sem RANGE_CLEAR behind a single
    DVE->Pool handshake; PE/Act/SP return immediately.

    Rationale: on NEFF return the runtime appends a per-engine semaphore
    reset train (~51 clears each, observed fixed mapping PE->S[2..53],
    Act->S[54..104], Pool->S[105..155], DVE->S[156..206], SP->S[207..255]).
    With the stock double all-engine barrier, every train runs after the
    last DMA lands, and PE's train (~6 us, slowest dispatch) sits on the
    critical path.  The only semaphores live at kernel end are the DMAHW
    sems 156-163 (incremented by in-flight DMA completions; in DVE's
    range) and the tile/barrier sems 151-163 cleared by Pool's RANGE_CLEAR
    and train.  PE/Act/SP's ranges are dead by the time their trains can
    start, so those engines may return while DMAs drain — their trains are
    hidden under the DMA window — provided DVE still waits for all DMA
    sems before returning and Pool's RANGE_CLEAR stays behind DVE's
    confirmation (the gather handshake)."""
    import concourse.mybir as mybir

    ET = mybir.EngineType
    end_blk = None
    for fn in nc.m.functions:
        for blk in fn.blocks:
            if blk.name.endswith("_end"):
                end_blk = blk
    assert end_blk is not None

    def waits(inst):
        si = inst.sync_info
        return list(si.on_wait) if si is not None and si.on_wait else []

    # Harvest the DMA-completion waits currently parked on SP.
    dma_waits = []
    for inst in end_blk.instructions:
        if inst.engine == ET.SP and isinstance(inst, (mybir.InstNoOp, mybir.InstDrain)):
            for w in waits(inst):
                if "DMAHW" in (w.ant_name or ""):
                    dma_waits.append(w)
    assert len(dma_waits) >= 4, [w.ant_name for w in dma_waits]

    seen = {ET.SP: 0, ET.Activation: 0, ET.PE: 0, ET.DVE: 0, ET.Pool: 0}
    out = []
    for inst in end_blk.instructions:
        eng = inst.engine
        if eng == ET.SP:
            if isinstance(inst, mybir.InstNoOp):
                continue  # waits harvested above
            if isinstance(inst, mybir.InstDrain) and seen[eng] == 0:
                seen[eng] += 1
                inst.sync_info = mybir.SyncInfo(on_wait=[], on_update=[])
                out.append(inst)  # plain pipeline flush, no waits
                continue
            continue  # remaining SP barrier insts dropped
        if eng == ET.Activation:
            if isinstance(inst, mybir.InstDrain) and seen[eng] == 0:
                seen[eng] += 1
                inst.sync_info = mybir.SyncInfo(on_wait=[], on_update=[])
                out.append(inst)
                continue
            continue
        if eng == ET.PE:
            continue  # no work, no barrier: return immediately
        if eng == ET.DVE:
            if isinstance(inst, mybir.InstDrain) and seen[eng] == 0:
                seen[eng] += 1
                # NoOps carrying all but the last DMA wait, then the Drain
                # waits the last one and signals Pool via the gather inc.
                for i, w in enumerate(dma_waits[:-1]):
                    nop = mybir.InstNoOp(name=f"dvew-{i}", ins=[], outs=[])
                    nop.engine = ET.DVE
                    nop.sync_info = mybir.SyncInfo(on_wait=[w], on_update=[])
                    out.append(nop)
                si = inst.sync_info
                ups = list(si.on_update) if si is not None and si.on_update else []
                assert any("gather" in (u.ant_name or "") for u in ups)
                inst.sync_info = mybir.SyncInfo(
                    on_wait=[dma_waits[-1]], on_update=ups)
                out.append(inst)
                continue
            continue  # barrier EventSemaphores + second round dropped
        if eng == ET.Pool:
            if isinstance(inst, mybir.InstEventSemaphore):
                si = inst.sync_info
                ws = waits(inst)
                if seen[eng] == 0 and ws and "gather" in (ws[0].ant_name or ""):
                    seen[eng] += 1
                    # only DVE increments now: wait 4 -> 1, sub 4 -> 1
                    ws[0].wait_value = 1
                    ups = list(si.on_update) if si and si.on_update else []
                    for u in ups:
                        if "gather" in (u.ant_name or ""):
                            u.update_value = 1
                    inst.sync_info = mybir.SyncInfo(on_wait=ws, on_update=ups)
                    out.append(inst)
                    continue
                continue  # release EventSemaphores + second round dropped
            if isinstance(inst, mybir.InstDrain):
                if seen[eng] >= 1 or not out or True:
                    # keep Pool drains up to the ISA range-clear; drop the
                    # trailing second-round drain (appears after the ISA)
                    if any(isinstance(x, mybir.InstISA) for x in out
                           if x.engine == ET.Pool):
                        continue
                    out.append(inst)
                    continue
            out.append(inst)  # the InstISA range-clear
            continue
        out.append(inst)
    end_blk.instructions = out


def _strip_const_memsets(nc):
    """Drop the framework's const-AP memsets (const-f32-0.0 etc.) from the
    entry block.  Nothing in this kernel reads those SBUF tiles, and they
    carry no sync info, so removing the writes cannot change any output."""
    import concourse.mybir as mybir

    for fn in nc.m.functions:
        for blk in fn.blocks:
            keep = []
            for inst in blk.instructions:
                if isinstance(inst, mybir.InstMemset):
                    outs = getattr(inst, "outs", [])
                    names = [getattr(o, "memref", "") or "" for o in outs]
                    si = inst.sync_info
                    no_sync = si is None or (not si.on_wait and not si.on_update)
                    if no_sync and names and all(n.startswith("const-") for n in names):
                        continue
                keep.append(inst)
            blk.instructions = keep


def _split_waits(nc, maxw=1):
    """This walrus build rejects instructions carrying more than one
    sync-wait ("Too many sync wait commands").  Tile can emit several on one
    instruction (notably the kernel-tail Drain).  Hoist the extras onto
    same-engine NoOps inserted immediately before the instruction."""
    import concourse.mybir as mybir

    nsplit = 0
    for fn in nc.m.functions:
        for blk in fn.blocks:
            newlist = []
            changed = False
            for inst in blk.instructions:
                si = inst.sync_info
                ow = list(si.on_wait) if si is not None and si.on_wait else []
                if len(ow) > maxw:
                    head, tail = ow[:-maxw], ow[-maxw:]
                    for w in head:
                        nop = mybir.InstNoOp(name=f"nopw-{nsplit}", ins=[], outs=[])
                        nsplit += 1
                        nop.engine = inst.engine
                        nop.sync_info = mybir.SyncInfo(on_wait=[w], on_update=[])
                        newlist.append(nop)
                    inst.sync_info = mybir.SyncInfo(
                        on_wait=tail,
                        on_update=list(si.on_update) if si.on_update else [])
                    changed = True
                newlist.append(inst)
            if changed:
                blk.instructions = newlist
    return nsplit


def _element_vals(params, sw_params, kinds, time):
    """Host replica of reference._element_vals (numpy, f32)."""
    params = np.asarray(params, dtype=np.float32)
    sw_params = np.asarray(sw_params, dtype=np.float32)
    kinds = np.asarray(kinds)
    t = int(time)
    sw_on = sw_params[:, t] > 0  # sigmoid(x) > 0.5  <=>  x > 0
    one = np.ones_like(params)
    zero = np.zeros_like(params)
    ndt = (np.float32(-DT) / params).astype(np.float32)
    z_vals = np.select(
        [kinds == 0, kinds == 1, kinds == 2, kinds == 3, kinds == 4, kinds == 5],
        [-params, zero, one, np.where(sw_on, 0.0, 1.0).astype(np.float32), ndt, one],
    ).astype(np.float32)
    y_vals = np.select(
        [kinds == 0, kinds == 1, kinds == 2, kinds == 3, kinds == 4, kinds == 5],
        [one, one, zero, np.where(sw_on, 1.0, 0.0).astype(np.float32), one, ndt],
    ).astype(np.float32)
    return z_vals, y_vals


def _run(M, params, sw_params, kinds, time, trace=False):
    from concourse.bass_utils import run_bass_kernel_spmd

    M = np.ascontiguousarray(np.asarray(M, dtype=np.float32))
    z_vals, y_vals = _element_vals(params, sw_params, kinds, time)
    negMt = -(M.T)  # [E, N] C-contiguous

    # [128, 2*RE] constant diagonal masks: [diag|0] then [0|diag]
    mask = np.ascontiguousarray(np.concatenate(
        [np.eye(128, RE, 0, dtype=np.float32),
         np.eye(128, RE, 128, dtype=np.float32)], axis=1))
    in_maps = []
    for c in range(C):
        # [128, 4] value columns (z0, z1, y0, y1): col k holds
        # vals[128*(k%2) + p] for this core's 256-element slice.
        zc = z_vals[RE * c:RE * (c + 1)].reshape(2, 128).T
        yc = y_vals[RE * c:RE * (c + 1)].reshape(2, 128).T
        in_maps.append({
            "mrow": M[RK * c:RK * (c + 1), :],
            "negmt": negMt[RE * c:RE * (c + 1), :],
            "vb": np.ascontiguousarray(np.concatenate([zc, yc], axis=1)),
            "mask": mask,
        })

    if "nc" not in _cache:
        _cache["nc"] = _build_nc()
    res = run_bass_kernel_spmd(
        _cache["nc"], in_maps, core_ids=list(range(C)), trace=trace,
        trace_cores=list(range(C)) if trace else None,
    )

    full = np.empty((N + 2 * E, 2 * E + N), dtype=np.float32)
    for c in range(C):
        r = res.results[c]
        om = r["out_main"]
        full[RK * c:RK * (c + 1), :] = om[0:RK]
        full[N + RE * c:N + RE * (c + 1), :] = om[RK:RK + RE]
        full[N + E + RE * c:N + E + RE * (c + 1), :] = om[RK + RE:RK + 2 * RE]
        # overlay core-dependent diagonal bands; out_bands is [128, 6*256]
        # with half-bands (i0 i1 z0 z1 y0 y1) along the free dim
        bands = r["out_bands"].reshape(128, 6, RE).transpose(1, 0, 2)
        ib = bands[0:2].reshape(RE, RE)
        zb = bands[2:4].reshape(RE, RE)
        yb = bands[4:6].reshape(RE, RE)
        full[N + RE * c:N + RE * (c + 1), E + RE * c:E + RE * (c + 1)] = ib
        full[N + E + RE * c:N + E + RE * (c + 1), RE * c:RE * (c + 1)] = zb
        full[N + E + RE * c:N + E + RE * (c + 1), E + RE * c:E + RE * (c + 1)] = yb
    return full, res


def kernel(M, params, sw_params, kinds, time):
    out, _ = _run(M, params, sw_params, kinds, time, trace=False)
    return out


# revision 12
# speedup vs baseline: 3.7063x; 1.2280x over previous
"""Trainium2 Bass kernel for nn_Coefficients: assemble the MNA coefficient
block matrix  [[M, 0, 0], [0, I, -M^T], [diag(z), diag(y), 0]]  of shape
[N+2E, 2E+N] from M [N,E], params/kinds/sw_params.

Sharding (8 cores, SPMD — one program, per-core data):
  core c owns kcl rows [128c,128c+128), kvl rows e in [256c,256c+256) and
  elem rows e in the same range.  Each core's out_main [640, 5120] holds its
  kcl/kvl/elem row chunks; out_bands [128, 6*256] holds the three 256x256
  diagonal blocks (identity, diag(z), diag(y)) packed as six 128x256
  half-bands, whose global column position depends on the core; the host
  unshard step places rows and overlays bands into the full [5120, 5120]
  output.

The PJRT execution path donates zero-initialised buffers as the kernel's
ExternalOutputs (see bass2jax.run_bass_via_pjrt zero_outs/donate_argnums —
kernels that don't write every element rely on that, and
test_bass2jax.py::test_donation guards it).  The structural-zero regions of
out_main therefore need no DMA traffic at all: the device writes only the
data-dependent bytes — the M row block, the -M^T block and the diagonal
bands — cutting per-core HBM traffic from ~15.9 MB to ~4.8 MB.

The toolchain allows only one sync-wait per instruction, so extra waits are
hoisted onto same-engine NoOps (_split_waits).
"""

import numpy as np

N, E, SIG = 1024, 2048, 64
C = 8            # cores
RK = N // C      # 128 kcl rows per core
RE = E // C      # 256 kvl/elem rows per core
W = 2 * E + N    # 5120 output width
DT = 1e-6

_cache = {}


def _build_nc():
    import concourse.bass as bass
    import concourse.mybir as mybir
    from concourse.tile import TileContext

    f32 = mybir.dt.float32
    nc = bass.Bass(name="coeffs_scatter", enable_partition_id=False)

    mrow = nc.dram_tensor("mrow", [RK, E], f32, kind="ExternalInput")
    negmt = nc.dram_tensor("negmt", [RE, N], f32, kind="ExternalInput")
    # Diagonal values [128, 4]: cols (z0, z1, y0, y1); col k holds
    # vals[128*(k%2) + p] at row p.
    vb = nc.dram_tensor("vb", [128, 4], f32, kind="ExternalInput")
    # Constant diagonal masks [128, 2*RE]: cols 0:RE = eye(128,RE,0)
    # ([diag|0]), cols RE:2*RE = eye(128,RE,128) ([0|diag]).  Serves as the
    # identity band content directly and as the multiplicand for the z/y
    # bands (the preloaded-constant idiom, like the PE-transpose identity).
    mask = nc.dram_tensor("mask", [128, 2 * RE], f32, kind="ExternalInput")

    out_main = nc.dram_tensor("out_main", [RK + 2 * RE, W], f32, kind="ExternalOutput")
    # Six [128, 256] half-bands (i0 i1 z0 z1 y0 y1) packed along the free
    # dim — SBUF layout dumped verbatim so the DMA gets 6 KB descriptors;
    # the host unpacks.
    out_bands = nc.dram_tensor("out_bands", [128, 6 * RE], f32, kind="ExternalOutput")

    with TileContext(nc) as tc:
        with tc.tile_pool(name="pool", bufs=1) as pool:
            # Loads on the SP ring: band values + diagonal masks.  (Hoisted
            # to the main block pre-barrier by _hoist_dmas_to_main.)
            vbt = pool.tile([128, 4], f32, tag="vbt")
            nc.sync.dma_start(out=vbt[:], in_=vb[:, :])
            mt = pool.tile([128, 2 * RE], f32, tag="mt")
            nc.sync.dma_start(out=mt[:], in_=mask[:, :])

            # ACT ring, no deps, off-peak: M rows into the kcl block, -M^T
            # rows into the kvl right block, and the identity bands (i0 i1),
            # which are exactly the mask content, DRAM->DRAM.
            nc.scalar.dma_start(out=out_main[0:RK, 0:E], in_=mrow[:, :])
            nc.scalar.dma_start(out=out_main[RK:RK + RE, 2 * E:W], in_=negmt[:, :])
            nc.scalar.dma_start(out=out_bands[:, 0:2 * RE], in_=mask[:, :])

            # z/y half-bands: mask half times per-partition value column.
            # Two multiplies per engine (Pool: z, DVE: y), in parallel.
            # One double-width multiply per engine: [z0|z1] = [mask0|mask1]
            # * [z0_col|z1_col] with the value columns broadcast along the
            # free dim via step-0 APs (Pool), same for [y0|y1] on DVE.
            bt = pool.tile([128, 4 * RE], f32, tag="bt")
            mm = mt[:, :].rearrange("p (k c) -> p k c", k=2)
            for j, eng in ((0, nc.gpsimd), (1, nc.vector)):
                vcols = vbt[:, 2 * j:2 * j + 2] \
                    .rearrange("p (k c) -> p k c", c=1) \
                    .broadcast_to([128, 2, RE])
                eng.tensor_mul(
                    bt[:, 2 * RE * j:2 * RE * (j + 1)]
                        .rearrange("p (k c) -> p k c", k=2),
                    mm, vcols,
                )

            # Band stores split across both (otherwise idle by now) rings,
            # each gated only on its own engine's multiply.
            nc.sync.dma_start(out=out_bands[:, 2 * RE:4 * RE], in_=bt[:, 0:2 * RE])
            nc.scalar.dma_start(out=out_bands[:, 4 * RE:], in_=bt[:, 2 * RE:])

    _strip_const_memsets(nc)
    _hoist_dmas_to_main(nc)
    _trim_end_barriers(nc)
    _split_waits(nc)
    return nc


def _hoist_dmas_to_main(nc):
    """Move the dependency-free DMA dispatches (vbt load, mrow, negmt) from
    the tile block into the entry block, before the head all-engine
    barrier, so their descriptors are generated ~0.7 us earlier and the
    bulk DRAM->DRAM traffic drains before the band chain needs HBM."""
    import concourse.mybir as mybir

    main_blk = None
    tile_blk = None
    for fn in nc.m.functions:
        for blk in fn.blocks:
            if blk.name == "main":
                main_blk = blk
            elif not blk.name.endswith("_end") and blk.name != "main":
                tile_blk = blk
    assert main_blk is not None and tile_blk is not None

    def waits(inst):
        si = inst.sync_info
        return list(si.on_wait) if si is not None and si.on_wait else []

    # dependency-free DMA copies only (no on_wait)
    hoist = [i for i in tile_blk.instructions
             if isinstance(i, mybir.InstDMACopy) and not waits(i)]
    tile_blk.instructions = [i for i in tile_blk.instructions if i not in hoist]

    # insert each before its engine's first Drain (the head barrier)
    out = []
    placed = set()
    for inst in main_blk.instructions:
        if isinstance(inst, mybir.InstDrain):
            for h in hoist:
                if h.engine == inst.engine and id(h) not in placed:
                    out.append(h)
                    placed.add(id(h))
        out.append(inst)
    assert len(placed) == len(hoist), (len(placed), len(hoist))
    main_blk.instructions = out


def _trim_end_barriers(nc):
    """Restructure the kernel-end block so only DVE waits for DMA
    completion and only Pool runs the tile-sem RANGE_CLEAR behind a single
    DVE->Pool handshake; PE/Act/SP return immediately.

    Rationale: on NEFF return the runtime appends a per-engine semaphore
    reset train (~51 clears each, observed fixed mapping PE->S[2..53],
    Act->S[54..104], Pool->S[105..155], DVE->S[156..206], SP->S[207..255]).
    With the stock double all-engine barrier, every train runs after the
    last DMA lands, and PE's train (~6 us, slowest dispatch) sits on the
    critical path.  The only semaphores live at kernel end are the DMAHW
    sems 156-163 (incremented by in-flight DMA completions; in DVE's
    range) and the tile/barrier sems 151-163 cleared by Pool's RANGE_CLEAR
    and train.  PE/Act/SP's ranges are dead by the time their trains can
    start, so those engines may return while DMAs drain — their trains are
    hidden under the DMA window — provided DVE still waits for all DMA
    sems before returning and Pool's RANGE_CLEAR stays behind DVE's
    confirmation (the gather handshake)."""
    import concourse.mybir as mybir

    ET = mybir.EngineType
    end_blk = None
    for fn in nc.m.functions:
        for blk in fn.blocks:
            if blk.name.endswith("_end"):
                end_blk = blk
    assert end_blk is not None

    def waits(inst):
        si = inst.sync_info
        return list(si.on_wait) if si is not None and si.on_wait else []

    # Harvest the DMA-completion waits currently parked on SP.
    dma_waits = []
    for inst in end_blk.instructions:
        if inst.engine == ET.SP and isinstance(inst, (mybir.InstNoOp, mybir.InstDrain)):
            for w in waits(inst):
                if "DMAHW" in (w.ant_name or ""):
                    dma_waits.append(w)
    assert len(dma_waits) >= 4, [w.ant_name for w in dma_waits]

    seen = {ET.SP: 0, ET.Activation: 0, ET.PE: 0, ET.DVE: 0, ET.Pool: 0}
    out = []
    for inst in end_blk.instructions:
        eng = inst.engine
        if eng == ET.SP:
            if isinstance(inst, mybir.InstNoOp):
                continue  # waits harvested above
            if isinstance(inst, mybir.InstDrain) and seen[eng] == 0:
                seen[eng] += 1
                inst.sync_info = mybir.SyncInfo(on_wait=[], on_update=[])
                out.append(inst)  # plain pipeline flush, no waits
                continue
            continue  # remaining SP barrier insts dropped
        if eng == ET.Activation:
            if isinstance(inst, mybir.InstDrain) and seen[eng] == 0:
                seen[eng] += 1
                inst.sync_info = mybir.SyncInfo(on_wait=[], on_update=[])
                out.append(inst)
                continue
            continue
        if eng == ET.PE:
            continue  # no work, no barrier: return immediately
        if eng == ET.DVE:
            if isinstance(inst, mybir.InstDrain) and seen[eng] == 0:
                seen[eng] += 1
                # NoOps carrying all but the last DMA wait, then the Drain
                # waits the last one and signals Pool via the gather inc.
                for i, w in enumerate(dma_waits[:-1]):
                    nop = mybir.InstNoOp(name=f"dvew-{i}", ins=[], outs=[])
                    nop.engine = ET.DVE
                    nop.sync_info = mybir.SyncInfo(on_wait=[w], on_update=[])
                    out.append(nop)
                si = inst.sync_info
                ups = list(si.on_update) if si is not None and si.on_update else []
                assert any("gather" in (u.ant_name or "") for u in ups)
                inst.sync_info = mybir.SyncInfo(
                    on_wait=[dma_waits[-1]], on_update=ups)
                out.append(inst)
                continue
            continue  # barrier EventSemaphores + second round dropped
        if eng == ET.Pool:
            if isinstance(inst, mybir.InstEventSemaphore):
                si = inst.sync_info
                ws = waits(inst)
                if seen[eng] == 0 and ws and "gather" in (ws[0].ant_name or ""):
                    seen[eng] += 1
                    # only DVE increments now: wait 4 -> 1, sub 4 -> 1
                    ws[0].wait_value = 1
                    ups = list(si.on_update) if si and si.on_update else []
                    for u in ups:
                        if "gather" in (u.ant_name or ""):
                            u.update_value = 1
                    inst.sync_info = mybir.SyncInfo(on_wait=ws, on_update=ups)
                    out.append(inst)
                    continue
                continue  # release EventSemaphores + second round dropped
            if isinstance(inst, mybir.InstDrain):
                if seen[eng] >= 1 or not out or True:
                    # keep Pool drains up to the ISA range-clear; drop the
                    # trailing second-round drain (appears after the ISA)
                    if any(isinstance(x, mybir.InstISA) for x in out
                           if x.engine == ET.Pool):
                        continue
                    out.append(inst)
                    continue
            out.append(inst)  # the InstISA range-clear
            continue
        out.append(inst)
    end_blk.instructions = out


def _strip_const_memsets(nc):
    """Drop the framework's const-AP memsets (const-f32-0.0 etc.) from the
    entry block.  Nothing in this kernel reads those SBUF tiles, and they
    carry no sync info, so removing the writes cannot change any output."""
    import concourse.mybir as mybir

    for fn in nc.m.functions:
        for blk in fn.blocks:
            keep = []
            for inst in blk.instructions:
                if isinstance(inst, mybir.InstMemset):
                    outs = getattr(inst, "outs", [])
                    names = [getattr(o, "memref", "") or "" for o in outs]
                    si = inst.sync_info
                    no_sync = si is None or (not si.on_wait and not si.on_update)
                    if no_sync and names and all(n.startswith("const-") for n in names):
                        continue
                keep.append(inst)
            blk.instructions = keep


def _split_waits(nc, maxw=1):
    """This walrus build rejects instructions carrying more than one
    sync-wait ("Too many sync wait commands").  Tile can emit several on one
    instruction (notably the kernel-tail Drain).  Hoist the extras onto
    same-engine NoOps inserted immediately before the instruction."""
    import concourse.mybir as mybir

    nsplit = 0
    for fn in nc.m.functions:
        for blk in fn.blocks:
            newlist = []
            changed = False
            for inst in blk.instructions:
                si = inst.sync_info
                ow = list(si.on_wait) if si is not None and si.on_wait else []
                if len(ow) > maxw:
                    head, tail = ow[:-maxw], ow[-maxw:]
                    for w in head:
                        nop = mybir.InstNoOp(name=f"nopw-{nsplit}", ins=[], outs=[])
                        nsplit += 1
                        nop.engine = inst.engine
                        nop.sync_info = mybir.SyncInfo(on_wait=[w], on_update=[])
                        newlist.append(nop)
                    inst.sync_info = mybir.SyncInfo(
                        on_wait=tail,
                        on_update=list(si.on_update) if si.on_update else [])
                    changed = True
                newlist.append(inst)
            if changed:
                blk.instructions = newlist
    return nsplit


def _element_vals(params, sw_params, kinds, time):
    """Host replica of reference._element_vals (numpy, f32)."""
    params = np.asarray(params, dtype=np.float32)
    sw_params = np.asarray(sw_params, dtype=np.float32)
    kinds = np.asarray(kinds)
    t = int(time)
    sw_on = sw_params[:, t] > 0  # sigmoid(x) > 0.5  <=>  x > 0
    one = np.ones_like(params)
    zero = np.zeros_like(params)
    ndt = (np.float32(-DT) / params).astype(np.float32)
    z_vals = np.select(
        [kinds == 0, kinds == 1, kinds == 2, kinds == 3, kinds == 4, kinds == 5],
        [-params, zero, one, np.where(sw_on, 0.0, 1.0).astype(np.float32), ndt, one],
    ).astype(np.float32)
    y_vals = np.select(
        [kinds == 0, kinds == 1, kinds == 2, kinds == 3, kinds == 4, kinds == 5],
        [one, one, zero, np.where(sw_on, 1.0, 0.0).astype(np.float32), one, ndt],
    ).astype(np.float32)
    return z_vals, y_vals


def _run(M, params, sw_params, kinds, time, trace=False):
    from concourse.bass_utils import run_bass_kernel_spmd

    M = np.ascontiguousarray(np.asarray(M, dtype=np.float32))
    z_vals, y_vals = _element_vals(params, sw_params, kinds, time)
    negMt = -(M.T)  # [E, N] C-contiguous

    # [128, 2*RE] constant diagonal masks: [diag|0] then [0|diag]
    mask = np.ascontiguousarray(np.concatenate(
        [np.eye(128, RE, 0, dtype=np.float32),
         np.eye(128, RE, 128, dtype=np.float32)], axis=1))
    in_maps = []
    for c in range(C):
        # [128, 4] value columns (z0, z1, y0, y1): col k holds
        # vals[128*(k%2) + p] for this core's 256-element slice.
        zc = z_vals[RE * c:RE * (c + 1)].reshape(2, 128).T
        yc = y_vals[RE * c:RE * (c + 1)].reshape(2, 128).T
        in_maps.append({
            "mrow": M[RK * c:RK * (c + 1), :],
            "negmt": negMt[RE * c:RE * (c + 1), :],
            "vb": np.ascontiguousarray(np.concatenate([zc, yc], axis=1)),
            "mask": mask,
        })

    if "nc" not in _cache:
        _cache["nc"] = _build_nc()
    res = run_bass_kernel_spmd(
        _cache["nc"], in_maps, core_ids=list(range(C)), trace=trace,
        trace_cores=list(range(C)) if trace else None,
    )

    full = np.empty((N + 2 * E, 2 * E + N), dtype=np.float32)
    for c in range(C):
        r = res.results[c]
        om = r["out_main"]
        full[RK * c:RK * (c + 1), :] = om[0:RK]
        full[N + RE * c:N + RE * (c + 1), :] = om[RK:RK + RE]
        full[N + E + RE * c:N + E + RE * (c + 1), :] = om[RK + RE:RK + 2 * RE]
        # overlay core-dependent diagonal bands; out_bands is [128, 6*256]
        # with half-bands (i0 i1 z0 z1 y0 y1) along the free dim
        bands = r["out_bands"].reshape(128, 6, RE).transpose(1, 0, 2)
        ib = bands[0:2].reshape(RE, RE)
        zb = bands[2:4].reshape(RE, RE)
        yb = bands[4:6].reshape(RE, RE)
        full[N + RE * c:N + RE * (c + 1), E + RE * c:E + RE * (c + 1)] = ib
        full[N + E + RE * c:N + E + RE * (c + 1), RE * c:RE * (c + 1)] = zb
        full[N + E + RE * c:N + E + RE * (c + 1), E + RE * c:E + RE * (c + 1)] = yb
    return full, res


def kernel(M, params, sw_params, kinds, time):
    out, _ = _run(M, params, sw_params, kinds, time, trace=False)
    return out


# revision 13
# speedup vs baseline: 4.1354x; 1.1158x over previous
"""Trainium2 Bass kernel for nn_Coefficients: assemble the MNA coefficient
block matrix  [[M, 0, 0], [0, I, -M^T], [diag(z), diag(y), 0]]  of shape
[N+2E, 2E+N] from M [N,E], params/kinds/sw_params.

Sharding (8 cores, SPMD — one program, per-core data):
  core c owns kcl rows [128c,128c+128), kvl rows e in [256c,256c+256) and
  elem rows e in the same range.  Each core's out_main [640, 5120] holds its
  kcl/kvl/elem row chunks; out_bands [128, 6*256] holds the three 256x256
  diagonal blocks (identity, diag(z), diag(y)) packed as six 128x256
  half-bands, whose global column position depends on the core; the host
  unshard step places rows and overlays bands into the full [5120, 5120]
  output.

The PJRT execution path donates zero-initialised buffers as the kernel's
ExternalOutputs (see bass2jax.run_bass_via_pjrt zero_outs/donate_argnums —
kernels that don't write every element rely on that, and
test_bass2jax.py::test_donation guards it).  The structural-zero regions of
out_main therefore need no DMA traffic at all: the device writes only the
data-dependent bytes — the M row block, the -M^T block and the diagonal
bands — cutting per-core HBM traffic from ~15.9 MB to ~4.8 MB.

The toolchain allows only one sync-wait per instruction, so extra waits are
hoisted onto same-engine NoOps (_split_waits).
"""

import numpy as np

N, E, SIG = 1024, 2048, 64
C = 8            # cores
RK = N // C      # 128 kcl rows per core
RE = E // C      # 256 kvl/elem rows per core
W = 2 * E + N    # 5120 output width
DT = 1e-6

_cache = {}


def _build_nc():
    import concourse.bass as bass
    import concourse.mybir as mybir
    from concourse.tile import TileContext

    f32 = mybir.dt.float32
    nc = bass.Bass(name="coeffs_scatter", enable_partition_id=False)

    mrow = nc.dram_tensor("mrow", [RK, E], f32, kind="ExternalInput")
    negmt = nc.dram_tensor("negmt", [RE, N], f32, kind="ExternalInput")
    # Diagonal values [128, 4]: cols (z0, z1, y0, y1); col k holds
    # vals[128*(k%2) + p] at row p.
    vb = nc.dram_tensor("vb", [128, 4], f32, kind="ExternalInput")
    # Constant diagonal masks [128, 2*RE]: cols 0:RE = eye(128,RE,0)
    # ([diag|0]), cols RE:2*RE = eye(128,RE,128) ([0|diag]).  Serves as the
    # identity band content directly and as the multiplicand for the z/y
    # bands (the preloaded-constant idiom, like the PE-transpose identity).
    mask = nc.dram_tensor("mask", [128, 2 * RE], f32, kind="ExternalInput")

    out_main = nc.dram_tensor("out_main", [RK + 2 * RE, W], f32, kind="ExternalOutput")
    # Six [128, 256] half-bands (i0 i1 z0 z1 y0 y1) packed along the free
    # dim — SBUF layout dumped verbatim so the DMA gets 6 KB descriptors;
    # the host unpacks.
    out_bands = nc.dram_tensor("out_bands", [128, 6 * RE], f32, kind="ExternalOutput")

    with TileContext(nc) as tc:
        with tc.tile_pool(name="pool", bufs=1) as pool:
            # Loads on the SP ring: band values + diagonal masks.  (Hoisted
            # to the main block pre-barrier by _hoist_dmas_to_main.)
            vbt = pool.tile([128, 4], f32, tag="vbt")
            nc.sync.dma_start(out=vbt[:], in_=vb[:, :])
            mt = pool.tile([128, 2 * RE], f32, tag="mt")
            nc.sync.dma_start(out=mt[:], in_=mask[:, :])

            # ACT ring, no deps, off-peak: M rows into the kcl block, -M^T
            # rows into the kvl right block, and the identity bands (i0 i1),
            # which are exactly the mask content, DRAM->DRAM.
            nc.scalar.dma_start(out=out_main[0:RK, 0:E], in_=mrow[:, :])
            nc.scalar.dma_start(out=out_main[RK:RK + RE, 2 * E:W], in_=negmt[:, :])
            nc.scalar.dma_start(out=out_bands[:, 0:2 * RE], in_=mask[:, :])

            # z/y half-bands: mask half times per-partition value column.
            # Two multiplies per engine (Pool: z, DVE: y), in parallel.
            # One quad-width multiply on DVE (the fastest elementwise
            # engine): [z0 z1 y0 y1] = [m0 m1 m0 m1] * [v0 v1 v2 v3], with
            # the mask halves repeated via a step-0 middle dim and the
            # value columns broadcast along the free dim.
            bt = pool.tile([128, 4 * RE], f32, tag="bt")
            mm = mt[:, :].rearrange("p (j k c) -> p j k c", j=1, k=2) \
                .broadcast_to([128, 2, 2, RE])
            vcols = vbt[:, :].rearrange("p (j k c) -> p j k c", j=2, c=1) \
                .broadcast_to([128, 2, 2, RE])
            nc.vector.tensor_mul(
                bt[:, :].rearrange("p (j k c) -> p j k c", j=2, k=2),
                mm, vcols,
            )

            # Band stores split across both (otherwise idle by now) rings.
            nc.sync.dma_start(out=out_bands[:, 2 * RE:4 * RE], in_=bt[:, 0:2 * RE])
            nc.scalar.dma_start(out=out_bands[:, 4 * RE:], in_=bt[:, 2 * RE:])

    _strip_const_memsets(nc)
    _hoist_dmas_to_main(nc)
    _trim_end_barriers(nc)
    _split_waits(nc)
    return nc


def _hoist_dmas_to_main(nc):
    """Move the dependency-free DMA dispatches (vbt load, mrow, negmt) from
    the tile block into the entry block, before the head all-engine
    barrier, so their descriptors are generated ~0.7 us earlier and the
    bulk DRAM->DRAM traffic drains before the band chain needs HBM."""
    import concourse.mybir as mybir

    main_blk = None
    tile_blk = None
    for fn in nc.m.functions:
        for blk in fn.blocks:
            if blk.name == "main":
                main_blk = blk
            elif not blk.name.endswith("_end") and blk.name != "main":
                tile_blk = blk
    assert main_blk is not None and tile_blk is not None

    def waits(inst):
        si = inst.sync_info
        return list(si.on_wait) if si is not None and si.on_wait else []

    # dependency-free DMA copies only (no on_wait)
    hoist = [i for i in tile_blk.instructions
             if isinstance(i, mybir.InstDMACopy) and not waits(i)]
    tile_blk.instructions = [i for i in tile_blk.instructions if i not in hoist]

    # insert each before its engine's first Drain (the head barrier)
    out = []
    placed = set()
    for inst in main_blk.instructions:
        if isinstance(inst, mybir.InstDrain):
            for h in hoist:
                if h.engine == inst.engine and id(h) not in placed:
                    out.append(h)
                    placed.add(id(h))
        out.append(inst)
    assert len(placed) == len(hoist), (len(placed), len(hoist))
    main_blk.instructions = out


def _trim_end_barriers(nc):
    """Restructure the kernel-end block so only DVE waits for DMA
    completion and only Pool runs the tile-sem RANGE_CLEAR behind a single
    DVE->Pool handshake; PE/Act/SP return immediately.

    Rationale: on NEFF return the runtime appends a per-engine semaphore
    reset train (~51 clears each, observed fixed mapping PE->S[2..53],
    Act->S[54..104], Pool->S[105..155], DVE->S[156..206], SP->S[207..255]).
    With the stock double all-engine barrier, every train runs after the
    last DMA lands, and PE's train (~6 us, slowest dispatch) sits on the
    critical path.  The only semaphores live at kernel end are the DMAHW
    sems 156-163 (incremented by in-flight DMA completions; in DVE's
    range) and the tile/barrier sems 151-163 cleared by Pool's RANGE_CLEAR
    and train.  PE/Act/SP's ranges are dead by the time their trains can
    start, so those engines may return while DMAs drain — their trains are
    hidden under the DMA window — provided DVE still waits for all DMA
    sems before returning and Pool's RANGE_CLEAR stays behind DVE's
    confirmation (the gather handshake)."""
    import concourse.mybir as mybir

    ET = mybir.EngineType
    end_blk = None
    for fn in nc.m.functions:
        for blk in fn.blocks:
            if blk.name.endswith("_end"):
                end_blk = blk
    assert end_blk is not None

    def waits(inst):
        si = inst.sync_info
        return list(si.on_wait) if si is not None and si.on_wait else []

    # Harvest the DMA-completion waits currently parked on SP.
    dma_waits = []
    for inst in end_blk.instructions:
        if inst.engine == ET.SP and isinstance(inst, (mybir.InstNoOp, mybir.InstDrain)):
            for w in waits(inst):
                if "DMAHW" in (w.ant_name or ""):
                    dma_waits.append(w)
    assert len(dma_waits) >= 4, [w.ant_name for w in dma_waits]

    seen = {ET.SP: 0, ET.Activation: 0, ET.PE: 0, ET.DVE: 0, ET.Pool: 0}
    out = []
    for inst in end_blk.instructions:
        eng = inst.engine
        if eng == ET.SP:
            if isinstance(inst, mybir.InstNoOp):
                continue  # waits harvested above
            if isinstance(inst, mybir.InstDrain) and seen[eng] == 0:
                seen[eng] += 1
                inst.sync_info = mybir.SyncInfo(on_wait=[], on_update=[])
                out.append(inst)  # plain pipeline flush, no waits
                continue
            continue  # remaining SP barrier insts dropped
        if eng == ET.Activation:
            if isinstance(inst, mybir.InstDrain) and seen[eng] == 0:
                seen[eng] += 1
                inst.sync_info = mybir.SyncInfo(on_wait=[], on_update=[])
                out.append(inst)
                continue
            continue
        if eng == ET.PE:
            continue  # no work, no barrier: return immediately
        if eng == ET.DVE:
            if isinstance(inst, mybir.InstDrain) and seen[eng] == 0:
                seen[eng] += 1
                # NoOps carrying all but the last DMA wait, then the Drain
                # waits the last one and signals Pool via the gather inc.
                for i, w in enumerate(dma_waits[:-1]):
                    nop = mybir.InstNoOp(name=f"dvew-{i}", ins=[], outs=[])
                    nop.engine = ET.DVE
                    nop.sync_info = mybir.SyncInfo(on_wait=[w], on_update=[])
                    out.append(nop)
                si = inst.sync_info
                ups = list(si.on_update) if si is not None and si.on_update else []
                assert any("gather" in (u.ant_name or "") for u in ups)
                inst.sync_info = mybir.SyncInfo(
                    on_wait=[dma_waits[-1]], on_update=ups)
                out.append(inst)
                continue
            continue  # barrier EventSemaphores + second round dropped
        if eng == ET.Pool:
            if isinstance(inst, mybir.InstEventSemaphore):
                si = inst.sync_info
                ws = waits(inst)
                if seen[eng] == 0 and ws and "gather" in (ws[0].ant_name or ""):
                    seen[eng] += 1
                    # only DVE increments now: wait 4 -> 1, sub 4 -> 1
                    ws[0].wait_value = 1
                    ups = list(si.on_update) if si and si.on_update else []
                    for u in ups:
                        if "gather" in (u.ant_name or ""):
                            u.update_value = 1
                    inst.sync_info = mybir.SyncInfo(on_wait=ws, on_update=ups)
                    out.append(inst)
                    continue
                continue  # release EventSemaphores + second round dropped
            if isinstance(inst, mybir.InstDrain):
                if seen[eng] >= 1 or not out or True:
                    # keep Pool drains up to the ISA range-clear; drop the
                    # trailing second-round drain (appears after the ISA)
                    if any(isinstance(x, mybir.InstISA) for x in out
                           if x.engine == ET.Pool):
                        continue
                    out.append(inst)
                    continue
            out.append(inst)  # the InstISA range-clear
            continue
        out.append(inst)
    end_blk.instructions = out


def _strip_const_memsets(nc):
    """Drop the framework's const-AP memsets (const-f32-0.0 etc.) from the
    entry block.  Nothing in this kernel reads those SBUF tiles, and they
    carry no sync info, so removing the writes cannot change any output."""
    import concourse.mybir as mybir

    for fn in nc.m.functions:
        for blk in fn.blocks:
            keep = []
            for inst in blk.instructions:
                if isinstance(inst, mybir.InstMemset):
                    outs = getattr(inst, "outs", [])
                    names = [getattr(o, "memref", "") or "" for o in outs]
                    si = inst.sync_info
                    no_sync = si is None or (not si.on_wait and not si.on_update)
                    if no_sync and names and all(n.startswith("const-") for n in names):
                        continue
                keep.append(inst)
            blk.instructions = keep


def _split_waits(nc, maxw=1):
    """This walrus build rejects instructions carrying more than one
    sync-wait ("Too many sync wait commands").  Tile can emit several on one
    instruction (notably the kernel-tail Drain).  Hoist the extras onto
    same-engine NoOps inserted immediately before the instruction."""
    import concourse.mybir as mybir

    nsplit = 0
    for fn in nc.m.functions:
        for blk in fn.blocks:
            newlist = []
            changed = False
            for inst in blk.instructions:
                si = inst.sync_info
                ow = list(si.on_wait) if si is not None and si.on_wait else []
                if len(ow) > maxw:
                    head, tail = ow[:-maxw], ow[-maxw:]
                    for w in head:
                        nop = mybir.InstNoOp(name=f"nopw-{nsplit}", ins=[], outs=[])
                        nsplit += 1
                        nop.engine = inst.engine
                        nop.sync_info = mybir.SyncInfo(on_wait=[w], on_update=[])
                        newlist.append(nop)
                    inst.sync_info = mybir.SyncInfo(
                        on_wait=tail,
                        on_update=list(si.on_update) if si.on_update else [])
                    changed = True
                newlist.append(inst)
            if changed:
                blk.instructions = newlist
    return nsplit


def _element_vals(params, sw_params, kinds, time):
    """Host replica of reference._element_vals (numpy, f32)."""
    params = np.asarray(params, dtype=np.float32)
    sw_params = np.asarray(sw_params, dtype=np.float32)
    kinds = np.asarray(kinds)
    t = int(time)
    sw_on = sw_params[:, t] > 0  # sigmoid(x) > 0.5  <=>  x > 0
    one = np.ones_like(params)
    zero = np.zeros_like(params)
    ndt = (np.float32(-DT) / params).astype(np.float32)
    z_vals = np.select(
        [kinds == 0, kinds == 1, kinds == 2, kinds == 3, kinds == 4, kinds == 5],
        [-params, zero, one, np.where(sw_on, 0.0, 1.0).astype(np.float32), ndt, one],
    ).astype(np.float32)
    y_vals = np.select(
        [kinds == 0, kinds == 1, kinds == 2, kinds == 3, kinds == 4, kinds == 5],
        [one, one, zero, np.where(sw_on, 1.0, 0.0).astype(np.float32), one, ndt],
    ).astype(np.float32)
    return z_vals, y_vals


def _run(M, params, sw_params, kinds, time, trace=False):
    from concourse.bass_utils import run_bass_kernel_spmd

    M = np.ascontiguousarray(np.asarray(M, dtype=np.float32))
    z_vals, y_vals = _element_vals(params, sw_params, kinds, time)
    negMt = -(M.T)  # [E, N] C-contiguous

    # [128, 2*RE] constant diagonal masks: [diag|0] then [0|diag]
    mask = np.ascontiguousarray(np.concatenate(
        [np.eye(128, RE, 0, dtype=np.float32),
         np.eye(128, RE, 128, dtype=np.float32)], axis=1))
    in_maps = []
    for c in range(C):
        # [128, 4] value columns (z0, z1, y0, y1): col k holds
        # vals[128*(k%2) + p] for this core's 256-element slice.
        zc = z_vals[RE * c:RE * (c + 1)].reshape(2, 128).T
        yc = y_vals[RE * c:RE * (c + 1)].reshape(2, 128).T
        in_maps.append({
            "mrow": M[RK * c:RK * (c + 1), :],
            "negmt": negMt[RE * c:RE * (c + 1), :],
            "vb": np.ascontiguousarray(np.concatenate([zc, yc], axis=1)),
            "mask": mask,
        })

    if "nc" not in _cache:
        _cache["nc"] = _build_nc()
    res = run_bass_kernel_spmd(
        _cache["nc"], in_maps, core_ids=list(range(C)), trace=trace,
        trace_cores=list(range(C)) if trace else None,
    )

    full = np.empty((N + 2 * E, 2 * E + N), dtype=np.float32)
    for c in range(C):
        r = res.results[c]
        om = r["out_main"]
        full[RK * c:RK * (c + 1), :] = om[0:RK]
        full[N + RE * c:N + RE * (c + 1), :] = om[RK:RK + RE]
        full[N + E + RE * c:N + E + RE * (c + 1), :] = om[RK + RE:RK + 2 * RE]
        # overlay core-dependent diagonal bands; out_bands is [128, 6*256]
        # with half-bands (i0 i1 z0 z1 y0 y1) along the free dim
        bands = r["out_bands"].reshape(128, 6, RE).transpose(1, 0, 2)
        ib = bands[0:2].reshape(RE, RE)
        zb = bands[2:4].reshape(RE, RE)
        yb = bands[4:6].reshape(RE, RE)
        full[N + RE * c:N + RE * (c + 1), E + RE * c:E + RE * (c + 1)] = ib
        full[N + E + RE * c:N + E + RE * (c + 1), RE * c:RE * (c + 1)] = zb
        full[N + E + RE * c:N + E + RE * (c + 1), E + RE * c:E + RE * (c + 1)] = yb
    return full, res


def kernel(M, params, sw_params, kinds, time):
    out, _ = _run(M, params, sw_params, kinds, time, trace=False)
    return out


# revision 15
# speedup vs baseline: 4.5699x; 1.1051x over previous
"""Trainium2 Bass kernel for nn_Coefficients: assemble the MNA coefficient
block matrix  [[M, 0, 0], [0, I, -M^T], [diag(z), diag(y), 0]]  of shape
[N+2E, 2E+N] from M [N,E], params/kinds/sw_params.

Sharding (8 cores, SPMD — one program, per-core data):
  core c owns kcl rows [128c,128c+128), kvl rows e in [256c,256c+256) and
  elem rows e in the same range.  Each core's out_main [640, 5120] holds its
  kcl/kvl/elem row chunks; out_bands [128, 6*256] holds the three 256x256
  diagonal blocks (identity, diag(z), diag(y)) packed as six 128x256
  half-bands, whose global column position depends on the core; the host
  unshard step places rows and overlays bands into the full [5120, 5120]
  output.

The PJRT execution path donates zero-initialised buffers as the kernel's
ExternalOutputs (see bass2jax.run_bass_via_pjrt zero_outs/donate_argnums —
kernels that don't write every element rely on that, and
test_bass2jax.py::test_donation guards it).  The structural-zero regions of
out_main therefore need no DMA traffic at all: the device writes only the
data-dependent bytes — the M row block, the -M^T block and the diagonal
bands — cutting per-core HBM traffic from ~15.9 MB to ~4.8 MB.

The toolchain allows only one sync-wait per instruction, so extra waits are
hoisted onto same-engine NoOps (_split_waits).
"""

import numpy as np

N, E, SIG = 1024, 2048, 64
C = 8            # cores
RK = N // C      # 128 kcl rows per core
RE = E // C      # 256 kvl/elem rows per core
W = 2 * E + N    # 5120 output width
DT = 1e-6

_cache = {}


def _build_nc():
    import concourse.bass as bass
    import concourse.mybir as mybir
    from concourse.tile import TileContext

    f32 = mybir.dt.float32
    nc = bass.Bass(name="coeffs_scatter", enable_partition_id=False)

    mrow = nc.dram_tensor("mrow", [RK, E], f32, kind="ExternalInput")
    negmt = nc.dram_tensor("negmt", [RE, N], f32, kind="ExternalInput")
    # Diagonal values [128, 4]: cols (z0, z1, y0, y1); col k holds
    # vals[128*(k%2) + p] at row p.
    vb = nc.dram_tensor("vb", [128, 4], f32, kind="ExternalInput")
    # Constant diagonal masks [128, 2*RE]: cols 0:RE = eye(128,RE,0)
    # ([diag|0]), cols RE:2*RE = eye(128,RE,128) ([0|diag]).  Serves as the
    # identity band content directly and as the multiplicand for the z/y
    # bands (the preloaded-constant idiom, like the PE-transpose identity).
    mask = nc.dram_tensor("mask", [128, 2 * RE], f32, kind="ExternalInput")

    out_main = nc.dram_tensor("out_main", [RK + 2 * RE, W], f32, kind="ExternalOutput")
    # Six [128, 256] half-bands (i0 i1 z0 z1 y0 y1) packed along the free
    # dim — SBUF layout dumped verbatim so the DMA gets 6 KB descriptors;
    # the host unpacks.
    out_bands = nc.dram_tensor("out_bands", [128, 6 * RE], f32, kind="ExternalOutput")

    with TileContext(nc) as tc:
        with tc.tile_pool(name="pool", bufs=1) as pool:
            # All input-side DMAs on the SP ring, in FIFO order with the
            # mask load last (it is the only dependency of the multiply):
            # identity bands (exactly the mask content, DRAM->DRAM), M rows
            # into the kcl block, -M^T rows into the kvl right block, then
            # the band values and masks into SBUF.  (Hoisted to the main
            # block pre-barrier by _hoist_dmas_to_main.)
            nc.sync.dma_start(out=out_bands[:, 0:2 * RE], in_=mask[:, :])
            nc.sync.dma_start(out=out_main[0:RK, 0:E], in_=mrow[:, :])
            nc.sync.dma_start(out=out_main[RK:RK + RE, 2 * E:W], in_=negmt[:, :])
            vbt = pool.tile([128, 4], f32, tag="vbt")
            nc.sync.dma_start(out=vbt[:], in_=vb[:, :])
            mt = pool.tile([128, 2 * RE], f32, tag="mt")
            nc.sync.dma_start(out=mt[:], in_=mask[:, :])

            # z/y half-bands: mask half times per-partition value column.
            # Two multiplies per engine (Pool: z, DVE: y), in parallel.
            # One quad-width multiply on DVE (the fastest elementwise
            # engine): [z0 z1 y0 y1] = [m0 m1 m0 m1] * [v0 v1 v2 v3], with
            # the mask halves repeated via a step-0 middle dim and the
            # value columns broadcast along the free dim.
            bt = pool.tile([128, 4 * RE], f32, tag="bt")
            mm = mt[:, :].rearrange("p (j k c) -> p j k c", j=1, k=2) \
                .broadcast_to([128, 2, 2, RE])
            vcols = vbt[:, :].rearrange("p (j k c) -> p j k c", j=2, c=1) \
                .broadcast_to([128, 2, 2, RE])
            nc.vector.tensor_mul(
                bt[:, :].rearrange("p (j k c) -> p j k c", j=2, k=2),
                mm, vcols,
            )

            # Single band store on the otherwise-idle ACT ring.
            nc.scalar.dma_start(out=out_bands[:, 2 * RE:], in_=bt[:, :])

    _strip_const_memsets(nc)
    _hoist_dmas_to_main(nc)
    _trim_end_barriers(nc)
    _split_waits(nc)
    return nc


def _hoist_dmas_to_main(nc):
    """Move the dependency-free DMA dispatches (vbt load, mrow, negmt) from
    the tile block into the entry block, before the head all-engine
    barrier, so their descriptors are generated ~0.7 us earlier and the
    bulk DRAM->DRAM traffic drains before the band chain needs HBM."""
    import concourse.mybir as mybir

    main_blk = None
    tile_blk = None
    for fn in nc.m.functions:
        for blk in fn.blocks:
            if blk.name == "main":
                main_blk = blk
            elif not blk.name.endswith("_end") and blk.name != "main":
                tile_blk = blk
    assert main_blk is not None and tile_blk is not None

    def waits(inst):
        si = inst.sync_info
        return list(si.on_wait) if si is not None and si.on_wait else []

    # dependency-free DMA copies only (no on_wait)
    hoist = [i for i in tile_blk.instructions
             if isinstance(i, mybir.InstDMACopy) and not waits(i)]
    tile_blk.instructions = [i for i in tile_blk.instructions if i not in hoist]

    # insert each before its engine's first Drain (the head barrier)
    out = []
    placed = set()
    for inst in main_blk.instructions:
        if isinstance(inst, mybir.InstDrain):
            for h in hoist:
                if h.engine == inst.engine and id(h) not in placed:
                    out.append(h)
                    placed.add(id(h))
        out.append(inst)
    assert len(placed) == len(hoist), (len(placed), len(hoist))
    main_blk.instructions = out


def _trim_end_barriers(nc):
    """Restructure the kernel-end block so only DVE waits for DMA
    completion and only Pool runs the tile-sem RANGE_CLEAR behind a single
    DVE->Pool handshake; PE/Act/SP return immediately.

    Rationale: on NEFF return the runtime appends a per-engine semaphore
    reset train (~51 clears each, observed fixed mapping PE->S[2..53],
    Act->S[54..104], Pool->S[105..155], DVE->S[156..206], SP->S[207..255]).
    With the stock double all-engine barrier, every train runs after the
    last DMA lands, and PE's train (~6 us, slowest dispatch) sits on the
    critical path.  The only semaphores live at kernel end are the DMAHW
    sems 156-163 (incremented by in-flight DMA completions; in DVE's
    range) and the tile/barrier sems 151-163 cleared by Pool's RANGE_CLEAR
    and train.  PE/Act/SP's ranges are dead by the time their trains can
    start, so those engines may return while DMAs drain — their trains are
    hidden under the DMA window — provided DVE still waits for all DMA
    sems before returning and Pool's RANGE_CLEAR stays behind DVE's
    confirmation (the gather handshake)."""
    import concourse.mybir as mybir

    ET = mybir.EngineType
    end_blk = None
    for fn in nc.m.functions:
        for blk in fn.blocks:
            if blk.name.endswith("_end"):
                end_blk = blk
    assert end_blk is not None

    def waits(inst):
        si = inst.sync_info
        return list(si.on_wait) if si is not None and si.on_wait else []

    # Harvest the DMA-completion waits currently parked on SP.
    dma_waits = []
    for inst in end_blk.instructions:
        if inst.engine == ET.SP and isinstance(inst, (mybir.InstNoOp, mybir.InstDrain)):
            for w in waits(inst):
                if "DMAHW" in (w.ant_name or ""):
                    dma_waits.append(w)
    assert len(dma_waits) >= 4, [w.ant_name for w in dma_waits]

    seen = {ET.SP: 0, ET.Activation: 0, ET.PE: 0, ET.DVE: 0, ET.Pool: 0}
    out = []
    for inst in end_blk.instructions:
        eng = inst.engine
        if eng == ET.SP:
            if isinstance(inst, mybir.InstNoOp):
                continue  # waits harvested above
            if isinstance(inst, mybir.InstDrain) and seen[eng] == 0:
                seen[eng] += 1
                inst.sync_info = mybir.SyncInfo(on_wait=[], on_update=[])
                out.append(inst)  # plain pipeline flush, no waits
                continue
            continue  # remaining SP barrier insts dropped
        if eng == ET.Activation:
            if isinstance(inst, mybir.InstDrain) and seen[eng] == 0:
                seen[eng] += 1
                inst.sync_info = mybir.SyncInfo(on_wait=[], on_update=[])
                out.append(inst)
                continue
            continue
        if eng == ET.PE:
            continue  # no work, no barrier: return immediately
        if eng == ET.DVE:
            if isinstance(inst, mybir.InstDrain) and seen[eng] == 0:
                seen[eng] += 1
                # NoOps carrying all but the last DMA wait, then the Drain
                # waits the last one and signals Pool via the gather inc.
                for i, w in enumerate(dma_waits[:-1]):
                    nop = mybir.InstNoOp(name=f"dvew-{i}", ins=[], outs=[])
                    nop.engine = ET.DVE
                    nop.sync_info = mybir.SyncInfo(on_wait=[w], on_update=[])
                    out.append(nop)
                si = inst.sync_info
                ups = list(si.on_update) if si is not None and si.on_update else []
                assert any("gather" in (u.ant_name or "") for u in ups)
                inst.sync_info = mybir.SyncInfo(
                    on_wait=[dma_waits[-1]], on_update=ups)
                out.append(inst)
                continue
            continue  # barrier EventSemaphores + second round dropped
        if eng == ET.Pool:
            if isinstance(inst, mybir.InstEventSemaphore):
                si = inst.sync_info
                ws = waits(inst)
                if seen[eng] == 0 and ws and "gather" in (ws[0].ant_name or ""):
                    seen[eng] += 1
                    # only DVE increments now: wait 4 -> 1, sub 4 -> 1
                    ws[0].wait_value = 1
                    ups = list(si.on_update) if si and si.on_update else []
                    for u in ups:
                        if "gather" in (u.ant_name or ""):
                            u.update_value = 1
                    inst.sync_info = mybir.SyncInfo(on_wait=ws, on_update=ups)
                    out.append(inst)
                    continue
                continue  # release EventSemaphores + second round dropped
            if isinstance(inst, mybir.InstDrain):
                if seen[eng] >= 1 or not out or True:
                    # keep Pool drains up to the ISA range-clear; drop the
                    # trailing second-round drain (appears after the ISA)
                    if any(isinstance(x, mybir.InstISA) for x in out
                           if x.engine == ET.Pool):
                        continue
                    out.append(inst)
                    continue
            out.append(inst)  # the InstISA range-clear
            continue
        out.append(inst)
    end_blk.instructions = out


def _strip_const_memsets(nc):
    """Drop the framework's const-AP memsets (const-f32-0.0 etc.) from the
    entry block.  Nothing in this kernel reads those SBUF tiles, and they
    carry no sync info, so removing the writes cannot change any output."""
    import concourse.mybir as mybir

    for fn in nc.m.functions:
        for blk in fn.blocks:
            keep = []
            for inst in blk.instructions:
                if isinstance(inst, mybir.InstMemset):
                    outs = getattr(inst, "outs", [])
                    names = [getattr(o, "memref", "") or "" for o in outs]
                    si = inst.sync_info
                    no_sync = si is None or (not si.on_wait and not si.on_update)
                    if no_sync and names and all(n.startswith("const-") for n in names):
                        continue
                keep.append(inst)
            blk.instructions = keep


def _split_waits(nc, maxw=1):
    """This walrus build rejects instructions carrying more than one
    sync-wait ("Too many sync wait commands").  Tile can emit several on one
    instruction (notably the kernel-tail Drain).  Hoist the extras onto
    same-engine NoOps inserted immediately before the instruction."""
    import concourse.mybir as mybir

    nsplit = 0
    for fn in nc.m.functions:
        for blk in fn.blocks:
            newlist = []
            changed = False
            for inst in blk.instructions:
                si = inst.sync_info
                ow = list(si.on_wait) if si is not None and si.on_wait else []
                if len(ow) > maxw:
                    head, tail = ow[:-maxw], ow[-maxw:]
                    for w in head:
                        nop = mybir.InstNoOp(name=f"nopw-{nsplit}", ins=[], outs=[])
                        nsplit += 1
                        nop.engine = inst.engine
                        nop.sync_info = mybir.SyncInfo(on_wait=[w], on_update=[])
                        newlist.append(nop)
                    inst.sync_info = mybir.SyncInfo(
                        on_wait=tail,
                        on_update=list(si.on_update) if si.on_update else [])
                    changed = True
                newlist.append(inst)
            if changed:
                blk.instructions = newlist
    return nsplit


def _element_vals(params, sw_params, kinds, time):
    """Host replica of reference._element_vals (numpy, f32)."""
    params = np.asarray(params, dtype=np.float32)
    sw_params = np.asarray(sw_params, dtype=np.float32)
    kinds = np.asarray(kinds)
    t = int(time)
    sw_on = sw_params[:, t] > 0  # sigmoid(x) > 0.5  <=>  x > 0
    one = np.ones_like(params)
    zero = np.zeros_like(params)
    ndt = (np.float32(-DT) / params).astype(np.float32)
    z_vals = np.select(
        [kinds == 0, kinds == 1, kinds == 2, kinds == 3, kinds == 4, kinds == 5],
        [-params, zero, one, np.where(sw_on, 0.0, 1.0).astype(np.float32), ndt, one],
    ).astype(np.float32)
    y_vals = np.select(
        [kinds == 0, kinds == 1, kinds == 2, kinds == 3, kinds == 4, kinds == 5],
        [one, one, zero, np.where(sw_on, 1.0, 0.0).astype(np.float32), one, ndt],
    ).astype(np.float32)
    return z_vals, y_vals


def _run(M, params, sw_params, kinds, time, trace=False):
    from concourse.bass_utils import run_bass_kernel_spmd

    M = np.ascontiguousarray(np.asarray(M, dtype=np.float32))
    z_vals, y_vals = _element_vals(params, sw_params, kinds, time)
    negMt = -(M.T)  # [E, N] C-contiguous

    # [128, 2*RE] constant diagonal masks: [diag|0] then [0|diag]
    mask = np.ascontiguousarray(np.concatenate(
        [np.eye(128, RE, 0, dtype=np.float32),
         np.eye(128, RE, 128, dtype=np.float32)], axis=1))
    in_maps = []
    for c in range(C):
        # [128, 4] value columns (z0, z1, y0, y1): col k holds
        # vals[128*(k%2) + p] for this core's 256-element slice.
        zc = z_vals[RE * c:RE * (c + 1)].reshape(2, 128).T
        yc = y_vals[RE * c:RE * (c + 1)].reshape(2, 128).T
        in_maps.append({
            "mrow": M[RK * c:RK * (c + 1), :],
            "negmt": negMt[RE * c:RE * (c + 1), :],
            "vb": np.ascontiguousarray(np.concatenate([zc, yc], axis=1)),
            "mask": mask,
        })

    if "nc" not in _cache:
        _cache["nc"] = _build_nc()
    res = run_bass_kernel_spmd(
        _cache["nc"], in_maps, core_ids=list(range(C)), trace=trace,
        trace_cores=list(range(C)) if trace else None,
    )

    full = np.empty((N + 2 * E, 2 * E + N), dtype=np.float32)
    for c in range(C):
        r = res.results[c]
        om = r["out_main"]
        full[RK * c:RK * (c + 1), :] = om[0:RK]
        full[N + RE * c:N + RE * (c + 1), :] = om[RK:RK + RE]
        full[N + E + RE * c:N + E + RE * (c + 1), :] = om[RK + RE:RK + 2 * RE]
        # overlay core-dependent diagonal bands; out_bands is [128, 6*256]
        # with half-bands (i0 i1 z0 z1 y0 y1) along the free dim
        bands = r["out_bands"].reshape(128, 6, RE).transpose(1, 0, 2)
        ib = bands[0:2].reshape(RE, RE)
        zb = bands[2:4].reshape(RE, RE)
        yb = bands[4:6].reshape(RE, RE)
        full[N + RE * c:N + RE * (c + 1), E + RE * c:E + RE * (c + 1)] = ib
        full[N + E + RE * c:N + E + RE * (c + 1), RE * c:RE * (c + 1)] = zb
        full[N + E + RE * c:N + E + RE * (c + 1), E + RE * c:E + RE * (c + 1)] = yb
    return full, res


def kernel(M, params, sw_params, kinds, time):
    out, _ = _run(M, params, sw_params, kinds, time, trace=False)
    return out
